# revision 4
# baseline (speedup 1.0000x reference)
"""Trainium2 Bass kernel for nn_BasicBlock (dense_cnn, active-shift block).

Data-parallel over batch: 32 images -> 4 per NeuronCore across 8 cores.
Per-core layout: channels on SBUF partitions, pixels (H*W) on the free dim.

Math restructure (validated vs the jax reference in fp32 to ~1e-7):
  - bn1+relu:  relu(s1*z + t1) = s1 * relu(z + t1/s1); the s1 scale is folded
    into the columns of w1, so bn1 is a single add+max tensor_scalar op
    (GpSimd), output in bf16.
  - conv1 (groups=2, bf16): two matmuls per pixel tile.  PE matmul outputs
    must start at partition 0 or 64, so the 96 fmap channels live interleaved
    on partitions [0:48] and [64:112]; partitions [48:64] are written zero via
    zero weight columns.  Everything after conv1 uses this padded
    112-partition layout (elementwise ops cost by free dim only, so the dead
    partitions are free); the fmap DMA and conv2 weights fold it back.
  - bn2+relu: ScalarE activation (per-partition scale/bias) from PSUM -> bf16.
  - active_shift is separable bilinear: a row pass (3 per-channel-weighted
    row-shifted MACs on VectorE, bf16) then a column pass folded into conv2's
    weights: conv2 becomes 3 matmuls with column-shifted access patterns.
  - conv2 (groups=3) is a block-diagonal matmul over the padded layout; the
    +x residual rides on the PSUM->SBUF eviction as a VectorE add against an
    on-chip reassembled contiguous copy of x.

Spatial tiling: 7 rows (392 px) per PSUM bank; pairs of banks share one PSUM
tile so bn2 / copies / residual adds run at 784-px granularity (amortizes the
per-instruction overheads).
"""

import os
import numpy as np
import ml_dtypes

import concourse.bass as bass
import concourse.bacc as bacc
import concourse.mybir as mybir
from concourse import tile
from concourse.bass_utils import run_bass_kernel_spmd

EPS = 1e-5
N_CORES = 8
N_PER = 4            # images per core
C = 96
CP = 112             # padded channel count for the post-conv1 layout
H = 56
W = 56
PIX = H * W          # 3136
RT = 7               # rows per spatial tile
TW = RT * W          # 392 pixels per tile (one PSUM bank each)
NT = H // RT         # 8 tiles per image
NPAIR = NT // 2      # 4 two-bank chunks per image
BANK = 512           # fp32 elems per PSUM bank

f32 = mybir.dt.float32
bf16 = mybir.dt.bfloat16

LAST_EXEC_NS = None


def _build_nc():
    nc = bacc.Bacc("TRN2", target_bir_lowering=False, debug=False)

    x_ext = nc.declare_dram_parameter("x", [N_PER, C, PIX], f32, isOutput=False)
    p_ext = nc.declare_dram_parameter("prev", [N_PER, C, PIX], f32, isOutput=False)
    bias1_ext = nc.declare_dram_parameter("bias1", [C, 2], f32, isOutput=False)
    s2_ext = nc.declare_dram_parameter("s2", [CP, 1], f32, isOutput=False)
    b2_ext = nc.declare_dram_parameter("b2", [CP, 1], f32, isOutput=False)
    w1t_ext = nc.declare_dram_parameter("w1t", [C, CP], bf16, isOutput=False)
    w2x_ext = nc.declare_dram_parameter("w2x", [CP, 288], bf16, isOutput=False)
    wr_ext = nc.declare_dram_parameter("wr", [CP, 3], f32, isOutput=False)
    out_ext = nc.declare_dram_parameter("out", [N_PER, C, PIX], f32, isOutput=True)
    fmap_ext = nc.declare_dram_parameter("fmap", [N_PER, C, PIX], f32, isOutput=True)

    with tile.TileContext(nc) as tc:
        with (
            tc.tile_pool(name="consts", bufs=1) as cpool,
            tc.tile_pool(name="raw", bufs=2) as rawp,
            tc.tile_pool(name="act", bufs=2) as actp,
            tc.tile_pool(name="bv", bufs=2) as bvp,
            tc.tile_pool(name="outs", bufs=2) as outp,
            tc.tile_pool(name="fpsum", bufs=2, space="PSUM") as fpsum,
            tc.tile_pool(name="opsum", bufs=2, space="PSUM") as opsum,
        ):
            w1_sb = cpool.tile([C, CP], bf16)
            nc.sync.dma_start(out=w1_sb[:], in_=w1t_ext[:])
            w2_sb = cpool.tile([CP, 288], bf16)
            nc.sync.dma_start(out=w2_sb[:], in_=w2x_ext[:])
            wr_sb = cpool.tile([CP, 3], f32)
            nc.sync.dma_start(out=wr_sb[:], in_=wr_ext[:])
            bias1_sb = cpool.tile([C, 2], f32)
            nc.sync.dma_start(out=bias1_sb[:], in_=bias1_ext[:])
            s2_sb = cpool.tile([CP, 1], f32)
            nc.sync.dma_start(out=s2_sb[:], in_=s2_ext[:])
            b2_sb = cpool.tile([CP, 1], f32)
            nc.sync.dma_start(out=b2_sb[:], in_=b2_ext[:])

            for n in range(N_PER):
                # group0 input = concat channels 0..95  = [x[0:48], prev[48:96]]
                # group1 input = concat channels 96..191 = [x[48:96], prev[0:48]]
                g0_raw = rawp.tile([C, PIX], f32, tag="g0raw")
                nc.sync.dma_start(out=g0_raw[0:48, :], in_=x_ext[n, 0:48, :])
                nc.sync.dma_start(out=g0_raw[48:96, :], in_=p_ext[n, 48:96, :])
                g1_raw = rawp.tile([C, PIX], f32, tag="g1raw")
                nc.sync.dma_start(out=g1_raw[0:48, :], in_=x_ext[n, 48:96, :])
                nc.sync.dma_start(out=g1_raw[48:96, :], in_=p_ext[n, 0:48, :])

                # contiguous copy of x for the residual (on-chip moves)
                xres = outp.tile([C, PIX], f32, tag="xres")
                nc.sync.dma_start(out=xres[0:48, :], in_=g0_raw[0:48, :])
                nc.sync.dma_start(out=xres[48:96, :], in_=g1_raw[0:48, :])

                # bn1 + relu (scale folded into w1): a = max(z + bias1, 0)
                g0_act = actp.tile([C, PIX], bf16, tag="g0act")
                nc.gpsimd.tensor_scalar(
                    g0_act[:], g0_raw[:], bias1_sb[:, 0:1], 0.0,
                    mybir.AluOpType.add, mybir.AluOpType.max,
                )
                g1_act = actp.tile([C, PIX], bf16, tag="g1act")
                nc.gpsimd.tensor_scalar(
                    g1_act[:], g1_raw[:], bias1_sb[:, 1:2], 0.0,
                    mybir.AluOpType.add, mybir.AluOpType.max,
                )

                b_sb = bvp.tile([CP, PIX], bf16, tag="b")
                v_sb = bvp.tile([CP, PIX], bf16, tag="v")
                fmap_sb = outp.tile([CP, PIX], f32, tag="fmap")
                out_sb = outp.tile([C, PIX], f32, tag="out")

                # conv1 (groups=2) + bn2(relu) + fmap eviction, per 2-bank chunk
                for cth in range(NPAIR):
                    fp = fpsum.tile([CP, 2 * BANK], f32, tag="fp")
                    for k in range(2):
                        t = 2 * cth + k
                        sl = slice(t * TW, (t + 1) * TW)
                        pb = slice(k * BANK, k * BANK + TW)
                        nc.tensor.matmul(
                            fp[0:64, pb], w1_sb[:, 0:64],
                            g0_act[:, sl], start=True, stop=True,
                        )
                        nc.tensor.matmul(
                            fp[64:112, pb], w1_sb[:, 64:112],
                            g1_act[:, sl], start=True, stop=True,
                        )
                    fpv = fp.rearrange("p (b w) -> p b w", w=BANK)[:, :, 0:TW]
                    csl = slice(cth * 2 * TW, (cth + 1) * 2 * TW)
                    bv = b_sb[:, csl].rearrange("p (b w) -> p b w", w=TW)
                    nc.scalar.activation(
                        bv, fpv, mybir.ActivationFunctionType.Relu,
                        bias=b2_sb[:, 0:1], scale=s2_sb[:, 0:1],
                    )
                    fv = fmap_sb[:, csl].rearrange("p (b w) -> p b w", w=TW)
                    nc.scalar.activation(
                        fv, fpv, mybir.ActivationFunctionType.Copy,
                    )

                nc.sync.dma_start(out=fmap_ext[n, 0:48, :], in_=fmap_sb[0:48, :])
                nc.sync.dma_start(out=fmap_ext[n, 48:96, :], in_=fmap_sb[64:112, :])

                # row pass of the shift: v[c,i,:] = sum_oy wr[c,oy]*b[c,i+oy,:]
                nc.vector.tensor_scalar(
                    v_sb[:, :], b_sb[:, :], wr_sb[:, 1:2], None,
                    mybir.AluOpType.mult,
                )
                nc.vector.scalar_tensor_tensor(
                    v_sb[:, W:PIX], b_sb[:, 0:PIX - W], wr_sb[:, 0:1],
                    v_sb[:, W:PIX], mybir.AluOpType.mult, mybir.AluOpType.add,
                )
                nc.vector.scalar_tensor_tensor(
                    v_sb[:, 0:PIX - W], b_sb[:, W:PIX], wr_sb[:, 2:3],
                    v_sb[:, 0:PIX - W], mybir.AluOpType.mult, mybir.AluOpType.add,
                )

                v3 = v_sb.rearrange("p (r w) -> p r w", w=W)

                # conv2 (col taps folded into weights) + residual on eviction
                for cth in range(NPAIR):
                    op = opsum.tile([C, 2 * BANK], f32, tag="op")
                    for k in range(2):
                        t = 2 * cth + k
                        sl = slice(t * TW, (t + 1) * TW)
                        pb = slice(k * BANK, k * BANK + TW)
                        r0 = t * RT
                        op3 = op[:, pb].rearrange("p (r w) -> p r w", w=W)
                        nc.tensor.matmul(
                            op[:, pb], w2_sb[:, 96:192], v_sb[:, sl],
                            start=True, stop=False, skip_group_check=True,
                        )
                        nc.tensor.matmul(
                            op3[:, :, 1:W], w2_sb[:, 0:96],
                            v3[:, r0:r0 + RT, 0:W - 1],
                            start=False, stop=False, skip_group_check=True,
                        )
                        nc.tensor.matmul(
                            op3[:, :, 0:W - 1], w2_sb[:, 192:288],
                            v3[:, r0:r0 + RT, 1:W],
                            start=False, stop=True, skip_group_check=True,
                        )
                    opv = op.rearrange("p (b w) -> p b w", w=BANK)[:, :, 0:TW]
                    csl = slice(cth * 2 * TW, (cth + 1) * 2 * TW)
                    ov = out_sb[:, csl].rearrange("p (b w) -> p b w", w=TW)
                    xv = xres[:, csl].rearrange("p (b w) -> p b w", w=TW)
                    nc.vector.tensor_tensor(
                        ov, opv, xv, mybir.AluOpType.add,
                    )

                nc.sync.dma_start(out=out_ext[n, :, :], in_=out_sb[:, :])

    nc.compile()
    return nc


def _prep_consts(bn1_gamma, bn1_beta, bn1_mean, bn1_var,
                 bn2_gamma, bn2_beta, bn2_mean, bn2_var, w1, w2, shift):
    s1 = bn1_gamma / np.sqrt(bn1_var + EPS)
    t1 = bn1_beta - bn1_mean * s1
    bias1 = (t1 / s1).astype(np.float32).reshape(2, C).T.copy()  # [96, 2]

    # padded index for original fmap channel c
    pidx = np.concatenate([np.arange(48), 64 + np.arange(48)])  # [96]

    s2f = bn2_gamma / np.sqrt(bn2_var + EPS)
    b2f = bn2_beta - bn2_mean * s2f
    s2 = np.zeros((CP, 1), np.float32)
    b2 = np.zeros((CP, 1), np.float32)
    s2[pidx, 0] = s2f
    b2[pidx, 0] = b2f

    w1m = w1[:, :, 0, 0]  # (96 out, 96 in-per-group)
    w1t = np.zeros((C, CP), np.float32)
    w1t[:, 0:48] = (w1m[0:48] * s1[None, 0:96]).T       # group0 lhsT [96K, 48M]
    w1t[:, 64:112] = (w1m[48:96] * s1[None, 96:192]).T  # group1 lhsT

    dy, dx = shift[:, 0].astype(np.float64), shift[:, 1].astype(np.float64)
    ay = np.floor(dy)
    ax = np.floor(dx)
    fy = dy - ay
    fx = dx - ax
    wrf = np.zeros((C, 3), np.float32)
    wcf = np.zeros((C, 3), np.float32)
    for c in range(C):
        iy = int(ay[c]) + 1   # -1 -> 0, 0 -> 1
        ix = int(ax[c]) + 1
        wrf[c, iy] += 1.0 - fy[c]
        wrf[c, iy + 1] += fy[c]
        wcf[c, ix] += 1.0 - fx[c]
        wcf[c, ix + 1] += fx[c]
    wr = np.zeros((CP, 3), np.float32)
    wr[pidx] = wrf

    w2m = w2[:, :, 0, 0]  # (96 out, 32 in-per-group)
    w2full = np.zeros((C, C), np.float32)
    for g in range(3):
        w2full[32 * g:32 * g + 32, 32 * g:32 * g + 32] = w2m[32 * g:32 * g + 32]
    w2x = np.zeros((CP, 288), np.float32)
    for k in range(3):
        # lhsT[pidx[c], o] = w2full[o, c] * wc[c, k]
        w2x[pidx, 96 * k:96 * k + 96] = w2full.T * wcf[:, k:k + 1]

    return {
        "bias1": bias1,
        "s2": s2,
        "b2": b2,
        "w1t": w1t.astype(ml_dtypes.bfloat16),
        "w2x": w2x.astype(ml_dtypes.bfloat16),
        "wr": wr,
    }


_NC_CACHE = {}


def kernel(x, prev_fmap, bn1_gamma, bn1_beta, bn1_mean, bn1_var,
           bn2_gamma, bn2_beta, bn2_mean, bn2_var, w1, w2, shift):
    global LAST_EXEC_NS
    x = np.ascontiguousarray(np.asarray(x, np.float32))
    prev_fmap = np.ascontiguousarray(np.asarray(prev_fmap, np.float32))
    consts = _prep_consts(
        np.asarray(bn1_gamma, np.float32), np.asarray(bn1_beta, np.float32),
        np.asarray(bn1_mean, np.float32), np.asarray(bn1_var, np.float32),
        np.asarray(bn2_gamma, np.float32), np.asarray(bn2_beta, np.float32),
        np.asarray(bn2_mean, np.float32), np.asarray(bn2_var, np.float32),
        np.asarray(w1, np.float32), np.asarray(w2, np.float32),
        np.asarray(shift, np.float32))

    if "nc" not in _NC_CACHE:
        _NC_CACHE["nc"] = _build_nc()
    nc = _NC_CACHE["nc"]

    NB = x.shape[0]
    xs = x.reshape(N_CORES, N_PER, C, PIX)
    ps = prev_fmap.reshape(N_CORES, N_PER, C, PIX)
    in_maps = [
        {"x": xs[i], "prev": ps[i], **consts}
        for i in range(N_CORES)
    ]

    trace = bool(os.environ.get("CC_KERNEL_TRACE"))
    res = run_bass_kernel_spmd(
        nc, in_maps, core_ids=list(range(N_CORES)), trace=trace,
    )
    LAST_EXEC_NS = res.exec_time_ns

    out = np.empty((NB, C, PIX), np.float32)
    fmap = np.empty((NB, C, PIX), np.float32)
    for i in range(N_CORES):
        out[i * N_PER:(i + 1) * N_PER] = res.results[i]["out"]
        fmap[i * N_PER:(i + 1) * N_PER] = res.results[i]["fmap"]
    return (out.reshape(NB, C, H, W), fmap.reshape(NB, C, H, W))


# revision 5
# speedup vs baseline: 3.8228x; 3.8228x over previous
"""Trainium2 Bass kernel for nn_BasicBlock (dense_cnn, active-shift block).

Data-parallel over batch: 32 images -> 4 per NeuronCore across 8 cores.
Per-core layout: channels on SBUF partitions, pixels (H*W) on the free dim.

Math restructure (validated vs the jax reference in fp32 to ~1e-7):
  - bn1+relu:  relu(s1*z + t1) = s1 * relu(z + t1/s1); the s1 scale is folded
    into the columns of w1, so bn1 is a single add+max tensor_scalar on
    VectorE (bf16, 4x mode).
  - conv1 (groups=2, bf16): two matmuls per pixel tile.  PE matmul outputs
    must start at partition 0 or 64, so the 96 fmap channels live interleaved
    on partitions [0:48] and [64:112]; partitions [48:64] are written zero via
    zero weight columns.  Everything after conv1 uses this padded
    112-partition layout (elementwise ops cost by free dim only, so the dead
    partitions are free); the fmap DMA and conv2 weights fold it back.
  - bn2+relu: ScalarE activation (per-partition scale/bias) from PSUM -> bf16.
  - active_shift is separable bilinear: a row pass on VectorE
    (v = wr0*b; bm = wrm*b; bp = wrp*b; v += shift(bm); v += shift(bp) --
    tensor_scalar 4x + tensor_tensor 2x only, no 1x-mode ops) and a column
    pass folded into conv2's weights (3 matmuls with column-shifted APs).
  - conv2 (groups=3) is a block-diagonal matmul over the padded layout; the
    +x residual is accumulated in PSUM via two shifted-identity matmuls from
    the bf16 raw tiles; ScalarE evicts the result.

dtype strategy: inputs are cast f32->bf16 by the load DMAs (GpSimd-initiated
casting DMAs; the GpSimd ALU pipeline stays empty -- its tensor ops are both
slow and poison concurrent VectorE ops via SBUF port sharing).  Outputs are
produced as bf16, DMA'd as bf16 (halves output HBM traffic) and widened to
f32 on the host.  End-to-end absmax-relative error ~3e-3.

Spatial tiling: 7 rows (392 px) per PSUM bank; pairs of banks share one PSUM
tile so bn2 / copies run at 784-px granularity (amortizes per-op overheads).
"""

import os
import numpy as np
import ml_dtypes

import concourse.bass as bass
import concourse.bacc as bacc
import concourse.mybir as mybir
from concourse import tile
from concourse.bass_utils import run_bass_kernel_spmd

EPS = 1e-5
N_CORES = 8
N_PER = 4            # images per core
C = 96
CP = 112             # padded channel count for the post-conv1 layout
H = 56
W = 56
PIX = H * W          # 3136
RT = 7               # rows per spatial tile
TW = RT * W          # 392 pixels per tile (one PSUM bank each)
NT = H // RT         # 8 tiles per image
NPAIR = NT // 2      # 4 two-bank chunks per image
BANK = 512           # fp32 elems per PSUM bank

f32 = mybir.dt.float32
bf16 = mybir.dt.bfloat16

LAST_EXEC_NS = None


def _build_nc():
    nc = bacc.Bacc("TRN2", target_bir_lowering=False, debug=False)

    x_ext = nc.declare_dram_parameter("x", [N_PER, C, PIX], f32, isOutput=False)
    p_ext = nc.declare_dram_parameter("prev", [N_PER, C, PIX], f32, isOutput=False)
    bias1_ext = nc.declare_dram_parameter("bias1", [C, 2], f32, isOutput=False)
    s2_ext = nc.declare_dram_parameter("s2", [CP, 1], f32, isOutput=False)
    b2_ext = nc.declare_dram_parameter("b2", [CP, 1], f32, isOutput=False)
    w1t_ext = nc.declare_dram_parameter("w1t", [C, CP], bf16, isOutput=False)
    w2x_ext = nc.declare_dram_parameter("w2x", [CP, 288], bf16, isOutput=False)
    wr_ext = nc.declare_dram_parameter("wr", [CP, 3], f32, isOutput=False)
    resw_ext = nc.declare_dram_parameter("resw", [C, 192], bf16, isOutput=False)
    out_ext = nc.declare_dram_parameter("out", [N_PER, C, PIX], bf16, isOutput=True)
    fmap_ext = nc.declare_dram_parameter("fmap", [N_PER, C, PIX], bf16, isOutput=True)

    with tile.TileContext(nc) as tc:
        with (
            tc.tile_pool(name="consts", bufs=1) as cpool,
            tc.tile_pool(name="raw", bufs=2) as rawp,
            tc.tile_pool(name="act", bufs=2) as actp,
            tc.tile_pool(name="bv", bufs=2) as bvp,
            tc.tile_pool(name="outs", bufs=2) as outp,
            tc.tile_pool(name="fpsum", bufs=2, space="PSUM") as fpsum,
            tc.tile_pool(name="opsum", bufs=2, space="PSUM") as opsum,
        ):
            w1_sb = cpool.tile([C, CP], bf16)
            nc.sync.dma_start(out=w1_sb[:], in_=w1t_ext[:])
            w2_sb = cpool.tile([CP, 288], bf16)
            nc.sync.dma_start(out=w2_sb[:], in_=w2x_ext[:])
            wr_sb = cpool.tile([CP, 3], f32)
            nc.sync.dma_start(out=wr_sb[:], in_=wr_ext[:])
            bias1_sb = cpool.tile([C, 2], f32)
            nc.sync.dma_start(out=bias1_sb[:], in_=bias1_ext[:])
            s2_sb = cpool.tile([CP, 1], f32)
            nc.sync.dma_start(out=s2_sb[:], in_=s2_ext[:])
            b2_sb = cpool.tile([CP, 1], f32)
            nc.sync.dma_start(out=b2_sb[:], in_=b2_ext[:])
            resw_sb = cpool.tile([C, 192], bf16)
            nc.sync.dma_start(out=resw_sb[:], in_=resw_ext[:])

            for n in range(N_PER):
                # group0 input = concat channels 0..95  = [x[0:48], prev[48:96]]
                # group1 input = concat channels 96..191 = [x[48:96], prev[0:48]]
                # casting DMAs (f32 -> bf16 in flight) must go via gpsimd rings
                g0_raw = rawp.tile([C, PIX], bf16, tag="g0raw")
                nc.gpsimd.dma_start(out=g0_raw[0:48, :], in_=x_ext[n, 0:48, :])
                nc.gpsimd.dma_start(out=g0_raw[48:96, :], in_=p_ext[n, 48:96, :])
                g1_raw = rawp.tile([C, PIX], bf16, tag="g1raw")
                nc.gpsimd.dma_start(out=g1_raw[0:48, :], in_=x_ext[n, 48:96, :])
                nc.gpsimd.dma_start(out=g1_raw[48:96, :], in_=p_ext[n, 0:48, :])

                # bn1 + relu (scale folded into w1): a = max(z + bias1, 0)
                g0_act = actp.tile([C, PIX], bf16, tag="g0act")
                nc.vector.tensor_scalar(
                    g0_act[:], g0_raw[:], bias1_sb[:, 0:1], 0.0,
                    mybir.AluOpType.add, mybir.AluOpType.max,
                )
                g1_act = actp.tile([C, PIX], bf16, tag="g1act")
                nc.vector.tensor_scalar(
                    g1_act[:], g1_raw[:], bias1_sb[:, 1:2], 0.0,
                    mybir.AluOpType.add, mybir.AluOpType.max,
                )

                b_sb = bvp.tile([CP, PIX], bf16, tag="b")
                v_sb = bvp.tile([CP, PIX], bf16, tag="v")
                bm_sb = bvp.tile([CP, PIX], bf16, tag="bm")
                bp_sb = bvp.tile([CP, PIX], bf16, tag="bp")
                fmap_sb = outp.tile([CP, PIX], bf16, tag="fmap")
                out_sb = outp.tile([C, PIX], bf16, tag="out")

                # conv1 (groups=2) + bn2(relu) + fmap eviction, per 2-bank chunk
                for cth in range(NPAIR):
                    fp = fpsum.tile([CP, 2 * BANK], f32, tag="fp")
                    for k in range(2):
                        t = 2 * cth + k
                        sl = slice(t * TW, (t + 1) * TW)
                        pb = slice(k * BANK, k * BANK + TW)
                        nc.tensor.matmul(
                            fp[0:64, pb], w1_sb[:, 0:64],
                            g0_act[:, sl], start=True, stop=True,
                        )
                        nc.tensor.matmul(
                            fp[64:112, pb], w1_sb[:, 64:112],
                            g1_act[:, sl], start=True, stop=True,
                        )
                    fpv = fp.rearrange("p (b w) -> p b w", w=BANK)[:, :, 0:TW]
                    csl = slice(cth * 2 * TW, (cth + 1) * 2 * TW)
                    bv = b_sb[:, csl].rearrange("p (b w) -> p b w", w=TW)
                    nc.scalar.activation(
                        bv, fpv, mybir.ActivationFunctionType.Relu,
                        bias=b2_sb[:, 0:1], scale=s2_sb[:, 0:1],
                    )
                    fv = fmap_sb[:, csl].rearrange("p (b w) -> p b w", w=TW)
                    nc.scalar.activation(
                        fv, fpv, mybir.ActivationFunctionType.Copy,
                    )

                nc.sync.dma_start(out=fmap_ext[n, 0:48, :], in_=fmap_sb[0:48, :])
                nc.sync.dma_start(out=fmap_ext[n, 48:96, :], in_=fmap_sb[64:112, :])

                # row pass of the shift: v[c,i,:] = sum_oy wr[c,oy]*b[c,i+oy,:]
                # tensor_scalar (4x) + tensor_tensor (2x) only; no 1x STT ops
                nc.vector.tensor_scalar(
                    v_sb[:, :], b_sb[:, :], wr_sb[:, 1:2], None,
                    mybir.AluOpType.mult,
                )
                nc.vector.tensor_scalar(
                    bm_sb[:, :], b_sb[:, :], wr_sb[:, 0:1], None,
                    mybir.AluOpType.mult,
                )
                nc.vector.tensor_scalar(
                    bp_sb[:, :], b_sb[:, :], wr_sb[:, 2:3], None,
                    mybir.AluOpType.mult,
                )
                nc.vector.tensor_tensor(
                    v_sb[:, W:PIX], bm_sb[:, 0:PIX - W], v_sb[:, W:PIX],
                    mybir.AluOpType.add,
                )
                nc.vector.tensor_tensor(
                    v_sb[:, 0:PIX - W], bp_sb[:, W:PIX], v_sb[:, 0:PIX - W],
                    mybir.AluOpType.add,
                )

                v3 = v_sb.rearrange("p (r w) -> p r w", w=W)

                # conv2 (col taps folded into weights) + residual, then evict
                for cth in range(NPAIR):
                    op = opsum.tile([C, 2 * BANK], f32, tag="op")
                    for k in range(2):
                        t = 2 * cth + k
                        sl = slice(t * TW, (t + 1) * TW)
                        pb = slice(k * BANK, k * BANK + TW)
                        r0 = t * RT
                        op3 = op[:, pb].rearrange("p (r w) -> p r w", w=W)
                        nc.tensor.matmul(
                            op[:, pb], w2_sb[:, 96:192], v_sb[:, sl],
                            start=True, stop=False, skip_group_check=True,
                        )
                        nc.tensor.matmul(
                            op3[:, :, 1:W], w2_sb[:, 0:96],
                            v3[:, r0:r0 + RT, 0:W - 1],
                            start=False, stop=False, skip_group_check=True,
                        )
                        nc.tensor.matmul(
                            op3[:, :, 0:W - 1], w2_sb[:, 192:288],
                            v3[:, r0:r0 + RT, 1:W],
                            start=False, stop=False, skip_group_check=True,
                        )
                        nc.tensor.matmul(
                            op[:, pb], resw_sb[:, 0:96], g0_raw[:, sl],
                            start=False, stop=False, skip_group_check=True,
                        )
                        nc.tensor.matmul(
                            op[:, pb], resw_sb[:, 96:192], g1_raw[:, sl],
                            start=False, stop=True, skip_group_check=True,
                        )
                    opv = op.rearrange("p (b w) -> p b w", w=BANK)[:, :, 0:TW]
                    csl = slice(cth * 2 * TW, (cth + 1) * 2 * TW)
                    ov = out_sb[:, csl].rearrange("p (b w) -> p b w", w=TW)
                    nc.scalar.activation(
                        ov, opv, mybir.ActivationFunctionType.Copy,
                    )

                nc.sync.dma_start(out=out_ext[n, :, :], in_=out_sb[:, :])

    nc.compile()
    return nc


def _prep_consts(bn1_gamma, bn1_beta, bn1_mean, bn1_var,
                 bn2_gamma, bn2_beta, bn2_mean, bn2_var, w1, w2, shift):
    s1 = bn1_gamma / np.sqrt(bn1_var + EPS)
    t1 = bn1_beta - bn1_mean * s1
    bias1 = (t1 / s1).astype(np.float32).reshape(2, C).T.copy()  # [96, 2]

    # padded index for original fmap channel c
    pidx = np.concatenate([np.arange(48), 64 + np.arange(48)])  # [96]

    s2f = bn2_gamma / np.sqrt(bn2_var + EPS)
    b2f = bn2_beta - bn2_mean * s2f
    s2 = np.zeros((CP, 1), np.float32)
    b2 = np.zeros((CP, 1), np.float32)
    s2[pidx, 0] = s2f
    b2[pidx, 0] = b2f

    w1m = w1[:, :, 0, 0]  # (96 out, 96 in-per-group)
    w1t = np.zeros((C, CP), np.float32)
    w1t[:, 0:48] = (w1m[0:48] * s1[None, 0:96]).T       # group0 lhsT [96K, 48M]
    w1t[:, 64:112] = (w1m[48:96] * s1[None, 96:192]).T  # group1 lhsT

    dy, dx = shift[:, 0].astype(np.float64), shift[:, 1].astype(np.float64)
    ay = np.floor(dy)
    ax = np.floor(dx)
    fy = dy - ay
    fx = dx - ax
    wrf = np.zeros((C, 3), np.float32)
    wcf = np.zeros((C, 3), np.float32)
    for c in range(C):
        iy = int(ay[c]) + 1   # -1 -> 0, 0 -> 1
        ix = int(ax[c]) + 1
        wrf[c, iy] += 1.0 - fy[c]
        wrf[c, iy + 1] += fy[c]
        wcf[c, ix] += 1.0 - fx[c]
        wcf[c, ix + 1] += fx[c]
    wr = np.zeros((CP, 3), np.float32)
    wr[pidx] = wrf

    w2m = w2[:, :, 0, 0]  # (96 out, 32 in-per-group)
    w2full = np.zeros((C, C), np.float32)
    for g in range(3):
        w2full[32 * g:32 * g + 32, 32 * g:32 * g + 32] = w2m[32 * g:32 * g + 32]
    w2x = np.zeros((CP, 288), np.float32)
    for k in range(3):
        # lhsT[pidx[c], o] = w2full[o, c] * wc[c, k]
        w2x[pidx, 96 * k:96 * k + 96] = w2full.T * wcf[:, k:k + 1]

    # residual: out[m] += g0_raw[m] (m<48) ; out[m] += g1_raw[m-48] (m>=48)
    resw = np.zeros((C, 192), np.float32)
    resw[np.arange(48), np.arange(48)] = 1.0            # from g0_raw -> m=k
    resw[np.arange(48), 96 + 48 + np.arange(48)] = 1.0  # from g1_raw -> m=k+48

    return {
        "bias1": bias1,
        "s2": s2,
        "b2": b2,
        "w1t": w1t.astype(ml_dtypes.bfloat16),
        "w2x": w2x.astype(ml_dtypes.bfloat16),
        "wr": wr,
        "resw": resw.astype(ml_dtypes.bfloat16),
    }


_NC_CACHE = {}


def kernel(x, prev_fmap, bn1_gamma, bn1_beta, bn1_mean, bn1_var,
           bn2_gamma, bn2_beta, bn2_mean, bn2_var, w1, w2, shift):
    global LAST_EXEC_NS
    x = np.ascontiguousarray(np.asarray(x, np.float32))
    prev_fmap = np.ascontiguousarray(np.asarray(prev_fmap, np.float32))
    consts = _prep_consts(
        np.asarray(bn1_gamma, np.float32), np.asarray(bn1_beta, np.float32),
        np.asarray(bn1_mean, np.float32), np.asarray(bn1_var, np.float32),
        np.asarray(bn2_gamma, np.float32), np.asarray(bn2_beta, np.float32),
        np.asarray(bn2_mean, np.float32), np.asarray(bn2_var, np.float32),
        np.asarray(w1, np.float32), np.asarray(w2, np.float32),
        np.asarray(shift, np.float32))

    if "nc" not in _NC_CACHE:
        _NC_CACHE["nc"] = _build_nc()
    nc = _NC_CACHE["nc"]

    NB = x.shape[0]
    xs = x.reshape(N_CORES, N_PER, C, PIX)
    ps = prev_fmap.reshape(N_CORES, N_PER, C, PIX)
    in_maps = [
        {"x": xs[i], "prev": ps[i], **consts}
        for i in range(N_CORES)
    ]

    trace = bool(os.environ.get("CC_KERNEL_TRACE"))
    res = run_bass_kernel_spmd(
        nc, in_maps, core_ids=list(range(N_CORES)), trace=trace,
    )
    LAST_EXEC_NS = res.exec_time_ns

    out = np.empty((NB, C, PIX), np.float32)
    fmap = np.empty((NB, C, PIX), np.float32)
    for i in range(N_CORES):
        out[i * N_PER:(i + 1) * N_PER] = res.results[i]["out"].astype(np.float32)
        fmap[i * N_PER:(i + 1) * N_PER] = res.results[i]["fmap"].astype(np.float32)
    return (out.reshape(NB, C, H, W), fmap.reshape(NB, C, H, W))


# revision 7
# speedup vs baseline: 5.1561x; 1.3488x over previous
"""Trainium2 Bass kernel for nn_BasicBlock (dense_cnn, active-shift block).

Data-parallel over batch: 32 images -> 4 per NeuronCore across 8 cores.
Per-core layout: channels on SBUF partitions, pixels (H*W) on the free dim.

Math restructure (validated vs the jax reference in fp32 to ~1e-7):
  - bn1+relu:  relu(s1*z + t1) = s1 * relu(z + t1/s1); the s1 scale is folded
    into the columns of w1, so bn1 is a single add+max tensor_scalar on
    VectorE (bf16, 4x mode).
  - conv1 (groups=2, bf16): two matmuls per pixel tile.  PE matmul outputs
    must start at partition 0 or 64, so the 96 fmap channels live interleaved
    on partitions [0:48] and [64:112]; partitions [48:64] are written zero via
    zero weight columns.  Everything after conv1 uses this padded
    112-partition layout (elementwise ops cost by free dim only, so the dead
    partitions are free); the fmap DMA and conv2 weights fold it back.
  - bn2+relu: ScalarE activation (per-partition scale/bias) from PSUM -> bf16.
  - active_shift is separable bilinear: a row pass on VectorE
    (v = wr0*b; bm = wrm*b; bp = wrp*b; v += shift(bm); v += shift(bp) --
    tensor_scalar 4x + tensor_tensor 2x only, no 1x-mode ops) and a column
    pass folded into conv2's weights (3 matmuls with column-shifted APs).
  - conv2 (groups=3) is a block-diagonal matmul over the padded layout; the
    +x residual is accumulated in PSUM via two shifted-identity matmuls from
    the bf16 raw tiles; ScalarE evicts the result.

dtype strategy: inputs are cast f32->bf16 by the load DMAs (GpSimd-initiated
casting DMAs; the GpSimd ALU pipeline stays empty -- its tensor ops are both
slow and poison concurrent VectorE ops via SBUF port sharing).  Outputs are
produced as bf16, DMA'd as bf16 (halves output HBM traffic) and widened to
f32 on the host.  End-to-end absmax-relative error ~3e-3.

Spatial tiling: 7 rows (392 px) per PSUM bank; pairs of banks share one PSUM
tile so bn2 / copies run at 784-px granularity (amortizes per-op overheads).
"""

import os
import numpy as np
import ml_dtypes

import concourse.bass as bass
import concourse.bacc as bacc
import concourse.mybir as mybir
from concourse import tile
from concourse.bass_utils import run_bass_kernel_spmd

EPS = 1e-5
N_CORES = 8
N_PER = 4            # images per core
C = 96
CP = 112             # padded channel count for the post-conv1 layout
H = 56
W = 56
PIX = H * W          # 3136
RT = 7               # rows per spatial tile
TW = RT * W          # 392 pixels per tile (one PSUM bank each)
NT = H // RT         # 8 tiles per image
NPAIR = NT // 2      # 4 two-bank chunks per image
BANK = 512           # fp32 elems per PSUM bank

f32 = mybir.dt.float32
bf16 = mybir.dt.bfloat16

LAST_EXEC_NS = None


def _build_nc():
    nc = bacc.Bacc("TRN2", target_bir_lowering=False, debug=False)

    x_ext = nc.declare_dram_parameter("x", [N_PER, C, PIX], f32, isOutput=False)
    p_ext = nc.declare_dram_parameter("prev", [N_PER, C, PIX], f32, isOutput=False)
    bias1_ext = nc.declare_dram_parameter("bias1", [C, 2], f32, isOutput=False)
    s2_ext = nc.declare_dram_parameter("s2", [CP, 1], f32, isOutput=False)
    b2_ext = nc.declare_dram_parameter("b2", [CP, 1], f32, isOutput=False)
    w1t_ext = nc.declare_dram_parameter("w1t", [C, CP], bf16, isOutput=False)
    w2x_ext = nc.declare_dram_parameter("w2x", [CP, 288], bf16, isOutput=False)
    wr_ext = nc.declare_dram_parameter("wr", [CP, 3], f32, isOutput=False)
    resw_ext = nc.declare_dram_parameter("resw", [C, 96], bf16, isOutput=False)
    out_ext = nc.declare_dram_parameter("out", [N_PER, C, PIX], bf16, isOutput=True)
    fmap_ext = nc.declare_dram_parameter("fmap", [N_PER, C, PIX], bf16, isOutput=True)

    with tile.TileContext(nc) as tc:
        with (
            tc.tile_pool(name="consts", bufs=1) as cpool,
            tc.tile_pool(name="raw", bufs=2) as rawp,
            tc.tile_pool(name="act", bufs=2) as actp,
            tc.tile_pool(name="bv", bufs=2) as bvp,
            tc.tile_pool(name="outs", bufs=2) as outp,
            tc.tile_pool(name="fpsum", bufs=2, space="PSUM") as fpsum,
            tc.tile_pool(name="opsum", bufs=2, space="PSUM") as opsum,
        ):
            w1_sb = cpool.tile([C, CP], bf16)
            nc.sync.dma_start(out=w1_sb[:], in_=w1t_ext[:])
            w2_sb = cpool.tile([CP, 288], bf16)
            nc.sync.dma_start(out=w2_sb[:], in_=w2x_ext[:])
            wr_sb = cpool.tile([CP, 3], f32)
            nc.sync.dma_start(out=wr_sb[:], in_=wr_ext[:])
            bias1_sb = cpool.tile([C, 2], f32)
            nc.sync.dma_start(out=bias1_sb[:], in_=bias1_ext[:])
            s2_sb = cpool.tile([CP, 1], f32)
            nc.sync.dma_start(out=s2_sb[:], in_=s2_ext[:])
            b2_sb = cpool.tile([CP, 1], f32)
            nc.sync.dma_start(out=b2_sb[:], in_=b2_ext[:])
            resw_sb = cpool.tile([C, 96], bf16)
            nc.sync.dma_start(out=resw_sb[:], in_=resw_ext[:])

            for n in range(N_PER):
                # group0 input = concat channels 0..95  = [x[0:48], prev[48:96]]
                # group1 input = concat channels 96..191 = [x[48:96], prev[0:48]]
                # casting DMAs (f32 -> bf16 in flight) must go via gpsimd rings
                g0_raw = rawp.tile([C, PIX], bf16, tag="g0raw")
                nc.gpsimd.dma_start(out=g0_raw[0:48, :], in_=x_ext[n, 0:48, :])
                nc.gpsimd.dma_start(out=g0_raw[48:96, :], in_=p_ext[n, 48:96, :])
                g1_raw = rawp.tile([C, PIX], bf16, tag="g1raw")
                nc.gpsimd.dma_start(out=g1_raw[0:48, :], in_=x_ext[n, 48:96, :])
                nc.gpsimd.dma_start(out=g1_raw[48:96, :], in_=p_ext[n, 0:48, :])

                # contiguous bf16 copy of x for the single-matmul residual
                xres = outp.tile([C, PIX], bf16, tag="xres")
                nc.sync.dma_start(out=xres[0:48, :], in_=g0_raw[0:48, :])
                nc.sync.dma_start(out=xres[48:96, :], in_=g1_raw[0:48, :])

                # bn1 + relu (scale folded into w1): a = max(z + bias1, 0)
                g0_act = actp.tile([C, PIX], bf16, tag="g0act")
                nc.vector.tensor_scalar(
                    g0_act[:], g0_raw[:], bias1_sb[:, 0:1], 0.0,
                    mybir.AluOpType.add, mybir.AluOpType.max,
                )
                g1_act = actp.tile([C, PIX], bf16, tag="g1act")
                nc.vector.tensor_scalar(
                    g1_act[:], g1_raw[:], bias1_sb[:, 1:2], 0.0,
                    mybir.AluOpType.add, mybir.AluOpType.max,
                )

                b_sb = bvp.tile([CP, PIX], bf16, tag="b")
                v_sb = bvp.tile([CP, PIX], bf16, tag="v")
                bm_sb = bvp.tile([CP, PIX], bf16, tag="bm")
                bp_sb = bvp.tile([CP, PIX], bf16, tag="bp")
                fmap_sb = outp.tile([CP, PIX], bf16, tag="fmap")
                out_sb = outp.tile([C, PIX], bf16, tag="out")

                # conv1 (groups=2) + bn2(relu) + fmap eviction, per 2-bank chunk
                for cth in range(NPAIR):
                    fp = fpsum.tile([CP, 2 * BANK], f32, tag="fp")
                    for k in range(2):
                        t = 2 * cth + k
                        sl = slice(t * TW, (t + 1) * TW)
                        pb = slice(k * BANK, k * BANK + TW)
                        nc.tensor.matmul(
                            fp[0:64, pb], w1_sb[:, 0:64],
                            g0_act[:, sl], start=True, stop=True,
                        )
                        nc.tensor.matmul(
                            fp[64:112, pb], w1_sb[:, 64:112],
                            g1_act[:, sl], start=True, stop=True,
                        )
                    fpv = fp.rearrange("p (b w) -> p b w", w=BANK)[:, :, 0:TW]
                    csl = slice(cth * 2 * TW, (cth + 1) * 2 * TW)
                    bv = b_sb[:, csl].rearrange("p (b w) -> p b w", w=TW)
                    nc.scalar.activation(
                        bv, fpv, mybir.ActivationFunctionType.Relu,
                        bias=b2_sb[:, 0:1], scale=s2_sb[:, 0:1],
                    )
                    fv = fmap_sb[:, csl].rearrange("p (b w) -> p b w", w=TW)
                    nc.scalar.activation(
                        fv, fpv, mybir.ActivationFunctionType.Copy,
                    )

                nc.sync.dma_start(out=fmap_ext[n, 0:48, :], in_=fmap_sb[0:48, :])
                nc.sync.dma_start(out=fmap_ext[n, 48:96, :], in_=fmap_sb[64:112, :])

                # row pass of the shift: v[c,i,:] = sum_oy wr[c,oy]*b[c,i+oy,:]
                # tensor_scalar (4x) + tensor_tensor (2x) only; no 1x STT ops.
                # Two halves, with the cross-half halo rows handled in the
                # second batch so every read refers to already-written data.
                HALF = PIX // 2
                for h0, h1 in ((0, HALF), (HALF, PIX)):
                    hs = slice(h0, h1)
                    nc.vector.tensor_scalar(
                        v_sb[:, hs], b_sb[:, hs], wr_sb[:, 1:2], None,
                        mybir.AluOpType.mult,
                    )
                    nc.vector.tensor_scalar(
                        bm_sb[:, hs], b_sb[:, hs], wr_sb[:, 0:1], None,
                        mybir.AluOpType.mult,
                    )
                    nc.vector.tensor_scalar(
                        bp_sb[:, hs], b_sb[:, hs], wr_sb[:, 2:3], None,
                        mybir.AluOpType.mult,
                    )
                    if h0 == 0:
                        # rows 1..27: bm rows 0..26 ; rows 0..26: bp rows 1..27
                        nc.vector.tensor_tensor(
                            v_sb[:, W:HALF], bm_sb[:, 0:HALF - W], v_sb[:, W:HALF],
                            mybir.AluOpType.add,
                        )
                        nc.vector.tensor_tensor(
                            v_sb[:, 0:HALF - W], bp_sb[:, W:HALF], v_sb[:, 0:HALF - W],
                            mybir.AluOpType.add,
                        )
                    else:
                        # rows 28..55: bm rows 27..54 ; rows 27..54: bp rows 28..55
                        nc.vector.tensor_tensor(
                            v_sb[:, HALF:PIX], bm_sb[:, HALF - W:PIX - W],
                            v_sb[:, HALF:PIX], mybir.AluOpType.add,
                        )
                        nc.vector.tensor_tensor(
                            v_sb[:, HALF - W:PIX - W], bp_sb[:, HALF:PIX],
                            v_sb[:, HALF - W:PIX - W], mybir.AluOpType.add,
                        )

                v3 = v_sb.rearrange("p (r w) -> p r w", w=W)

                # conv2 (col taps folded into weights) + residual, then evict
                for cth in range(NPAIR):
                    op = opsum.tile([C, 2 * BANK], f32, tag="op")
                    for k in range(2):
                        t = 2 * cth + k
                        sl = slice(t * TW, (t + 1) * TW)
                        pb = slice(k * BANK, k * BANK + TW)
                        r0 = t * RT
                        op3 = op[:, pb].rearrange("p (r w) -> p r w", w=W)
                        nc.tensor.matmul(
                            op[:, pb], w2_sb[:, 96:192], v_sb[:, sl],
                            start=True, stop=False, skip_group_check=True,
                        )
                        nc.tensor.matmul(
                            op3[:, :, 1:W], w2_sb[:, 0:96],
                            v3[:, r0:r0 + RT, 0:W - 1],
                            start=False, stop=False, skip_group_check=True,
                        )
                        nc.tensor.matmul(
                            op3[:, :, 0:W - 1], w2_sb[:, 192:288],
                            v3[:, r0:r0 + RT, 1:W],
                            start=False, stop=False, skip_group_check=True,
                        )
                        nc.tensor.matmul(
                            op[:, pb], resw_sb[:, 0:96], xres[:, sl],
                            start=False, stop=True, skip_group_check=True,
                        )
                    opv = op.rearrange("p (b w) -> p b w", w=BANK)[:, :, 0:TW]
                    csl = slice(cth * 2 * TW, (cth + 1) * 2 * TW)
                    ov = out_sb[:, csl].rearrange("p (b w) -> p b w", w=TW)
                    nc.scalar.activation(
                        ov, opv, mybir.ActivationFunctionType.Copy,
                    )

                nc.sync.dma_start(out=out_ext[n, :, :], in_=out_sb[:, :])

    nc.compile()
    return nc


def _prep_consts(bn1_gamma, bn1_beta, bn1_mean, bn1_var,
                 bn2_gamma, bn2_beta, bn2_mean, bn2_var, w1, w2, shift):
    s1 = bn1_gamma / np.sqrt(bn1_var + EPS)
    t1 = bn1_beta - bn1_mean * s1
    bias1 = (t1 / s1).astype(np.float32).reshape(2, C).T.copy()  # [96, 2]

    # padded index for original fmap channel c
    pidx = np.concatenate([np.arange(48), 64 + np.arange(48)])  # [96]

    s2f = bn2_gamma / np.sqrt(bn2_var + EPS)
    b2f = bn2_beta - bn2_mean * s2f
    s2 = np.zeros((CP, 1), np.float32)
    b2 = np.zeros((CP, 1), np.float32)
    s2[pidx, 0] = s2f
    b2[pidx, 0] = b2f

    w1m = w1[:, :, 0, 0]  # (96 out, 96 in-per-group)
    w1t = np.zeros((C, CP), np.float32)
    w1t[:, 0:48] = (w1m[0:48] * s1[None, 0:96]).T       # group0 lhsT [96K, 48M]
    w1t[:, 64:112] = (w1m[48:96] * s1[None, 96:192]).T  # group1 lhsT

    dy, dx = shift[:, 0].astype(np.float64), shift[:, 1].astype(np.float64)
    ay = np.floor(dy)
    ax = np.floor(dx)
    fy = dy - ay
    fx = dx - ax
    wrf = np.zeros((C, 3), np.float32)
    wcf = np.zeros((C, 3), np.float32)
    for c in range(C):
        iy = int(ay[c]) + 1   # -1 -> 0, 0 -> 1
        ix = int(ax[c]) + 1
        wrf[c, iy] += 1.0 - fy[c]
        wrf[c, iy + 1] += fy[c]
        wcf[c, ix] += 1.0 - fx[c]
        wcf[c, ix + 1] += fx[c]
    wr = np.zeros((CP, 3), np.float32)
    wr[pidx] = wrf

    w2m = w2[:, :, 0, 0]  # (96 out, 32 in-per-group)
    w2full = np.zeros((C, C), np.float32)
    for g in range(3):
        w2full[32 * g:32 * g + 32, 32 * g:32 * g + 32] = w2m[32 * g:32 * g + 32]
    w2x = np.zeros((CP, 288), np.float32)
    for k in range(3):
        # lhsT[pidx[c], o] = w2full[o, c] * wc[c, k]
        w2x[pidx, 96 * k:96 * k + 96] = w2full.T * wcf[:, k:k + 1]

    # residual: identity matmul from the contiguous xres tile
    resw = np.eye(C, dtype=np.float32)

    return {
        "bias1": bias1,
        "s2": s2,
        "b2": b2,
        "w1t": w1t.astype(ml_dtypes.bfloat16),
        "w2x": w2x.astype(ml_dtypes.bfloat16),
        "wr": wr,
        "resw": resw.astype(ml_dtypes.bfloat16),
    }


_NC_CACHE = {}


def kernel(x, prev_fmap, bn1_gamma, bn1_beta, bn1_mean, bn1_var,
           bn2_gamma, bn2_beta, bn2_mean, bn2_var, w1, w2, shift):
    global LAST_EXEC_NS
    x = np.ascontiguousarray(np.asarray(x, np.float32))
    prev_fmap = np.ascontiguousarray(np.asarray(prev_fmap, np.float32))
    consts = _prep_consts(
        np.asarray(bn1_gamma, np.float32), np.asarray(bn1_beta, np.float32),
        np.asarray(bn1_mean, np.float32), np.asarray(bn1_var, np.float32),
        np.asarray(bn2_gamma, np.float32), np.asarray(bn2_beta, np.float32),
        np.asarray(bn2_mean, np.float32), np.asarray(bn2_var, np.float32),
        np.asarray(w1, np.float32), np.asarray(w2, np.float32),
        np.asarray(shift, np.float32))

    if "nc" not in _NC_CACHE:
        _NC_CACHE["nc"] = _build_nc()
    nc = _NC_CACHE["nc"]

    NB = x.shape[0]
    xs = x.reshape(N_CORES, N_PER, C, PIX)
    ps = prev_fmap.reshape(N_CORES, N_PER, C, PIX)
    in_maps = [
        {"x": xs[i], "prev": ps[i], **consts}
        for i in range(N_CORES)
    ]

    trace = bool(os.environ.get("CC_KERNEL_TRACE"))
    res = run_bass_kernel_spmd(
        nc, in_maps, core_ids=list(range(N_CORES)), trace=trace,
    )
    LAST_EXEC_NS = res.exec_time_ns

    out = np.empty((NB, C, PIX), np.float32)
    fmap = np.empty((NB, C, PIX), np.float32)
    for i in range(N_CORES):
        out[i * N_PER:(i + 1) * N_PER] = res.results[i]["out"].astype(np.float32)
        fmap[i * N_PER:(i + 1) * N_PER] = res.results[i]["fmap"].astype(np.float32)
    return (out.reshape(NB, C, H, W), fmap.reshape(NB, C, H, W))


# revision 8
# speedup vs baseline: 5.4968x; 1.0661x over previous
"""Trainium2 Bass kernel for nn_BasicBlock (dense_cnn, active-shift block).

Data-parallel over batch: 32 images -> 4 per NeuronCore across 8 cores.
Per-core layout: channels on SBUF partitions, pixels (H*W) on the free dim.

Math restructure (validated vs the jax reference in fp32 to ~1e-7):
  - bn1+relu:  relu(s1*z + t1) = s1 * relu(z + t1/s1); the s1 scale is folded
    into the columns of w1, so bn1 is a single add+max tensor_scalar on
    VectorE (bf16, 4x mode).
  - conv1 (groups=2, bf16): two matmuls per pixel tile.  PE matmul outputs
    must start at partition 0 or 64, so the 96 fmap channels live interleaved
    on partitions [0:48] and [64:112]; partitions [48:64] are written zero via
    zero weight columns.  Everything after conv1 uses this padded
    112-partition layout (elementwise ops cost by free dim only, so the dead
    partitions are free); the fmap DMA and conv2 weights fold it back.
  - bn2+relu: ScalarE activation (per-partition scale/bias) from PSUM -> bf16.
  - active_shift is separable bilinear: a row pass on VectorE
    (v = wr0*b; bm = wrm*b; bp = wrp*b; v += shift(bm); v += shift(bp) --
    tensor_scalar 4x + tensor_tensor 2x only, no 1x-mode ops) and a column
    pass folded into conv2's weights (3 matmuls with column-shifted APs).
  - conv2 (groups=3) is a block-diagonal matmul over the padded layout; the
    +x residual is accumulated in PSUM via two shifted-identity matmuls from
    the bf16 raw tiles; ScalarE evicts the result.

dtype strategy: inputs are cast f32->bf16 by the load DMAs (GpSimd-initiated
casting DMAs; the GpSimd ALU pipeline stays empty -- its tensor ops are both
slow and poison concurrent VectorE ops via SBUF port sharing).  Outputs are
produced as bf16, DMA'd as bf16 (halves output HBM traffic) and widened to
f32 on the host.  End-to-end absmax-relative error ~3e-3.

Spatial tiling: 7 rows (392 px) per PSUM bank; pairs of banks share one PSUM
tile so bn2 / copies run at 784-px granularity (amortizes per-op overheads).
"""

import os
import numpy as np
import ml_dtypes

import concourse.bass as bass
import concourse.bacc as bacc
import concourse.mybir as mybir
from concourse import tile
from concourse.bass_utils import run_bass_kernel_spmd

EPS = 1e-5
N_CORES = 8
N_PER = 4            # images per core
C = 96
CP = 112             # padded channel count for the post-conv1 layout
H = 56
W = 56
PIX = H * W          # 3136
RT = 7               # rows per spatial tile
TW = RT * W          # 392 pixels per tile (one PSUM bank each)
NT = H // RT         # 8 tiles per image
NPAIR = NT // 2      # 4 two-bank chunks per image
BANK = 512           # fp32 elems per PSUM bank

f32 = mybir.dt.float32
bf16 = mybir.dt.bfloat16

LAST_EXEC_NS = None


def _build_nc():
    nc = bacc.Bacc("TRN2", target_bir_lowering=False, debug=False)

    x_ext = nc.declare_dram_parameter("x", [N_PER, C, PIX], f32, isOutput=False)
    p_ext = nc.declare_dram_parameter("prev", [N_PER, C, PIX], f32, isOutput=False)
    bias1_ext = nc.declare_dram_parameter("bias1", [C, 2], f32, isOutput=False)
    s2_ext = nc.declare_dram_parameter("s2", [CP, 1], f32, isOutput=False)
    b2_ext = nc.declare_dram_parameter("b2", [CP, 1], f32, isOutput=False)
    w1t_ext = nc.declare_dram_parameter("w1t", [C, CP], bf16, isOutput=False)
    w2x_ext = nc.declare_dram_parameter("w2x", [CP, 288], bf16, isOutput=False)
    wr_ext = nc.declare_dram_parameter("wr", [CP, 3], f32, isOutput=False)
    resw_ext = nc.declare_dram_parameter("resw", [C, 96], bf16, isOutput=False)
    out_ext = nc.declare_dram_parameter("out", [N_PER, C, PIX], bf16, isOutput=True)
    fmap_ext = nc.declare_dram_parameter("fmap", [N_PER, C, PIX], bf16, isOutput=True)

    with tile.TileContext(nc) as tc:
        with (
            tc.tile_pool(name="consts", bufs=1) as cpool,
            tc.tile_pool(name="raw", bufs=2) as rawp,
            tc.tile_pool(name="act", bufs=2) as actp,
            tc.tile_pool(name="bv", bufs=2) as bvp,
            tc.tile_pool(name="outs", bufs=2) as outp,
            tc.tile_pool(name="fpsum", bufs=2, space="PSUM") as fpsum,
            tc.tile_pool(name="opsum", bufs=2, space="PSUM") as opsum,
        ):
            w1_sb = cpool.tile([C, CP], bf16)
            nc.sync.dma_start(out=w1_sb[:], in_=w1t_ext[:])
            w2_sb = cpool.tile([CP, 288], bf16)
            nc.sync.dma_start(out=w2_sb[:], in_=w2x_ext[:])
            wr_sb = cpool.tile([CP, 3], f32)
            nc.sync.dma_start(out=wr_sb[:], in_=wr_ext[:])
            bias1_sb = cpool.tile([C, 2], f32)
            nc.sync.dma_start(out=bias1_sb[:], in_=bias1_ext[:])
            s2_sb = cpool.tile([CP, 1], f32)
            nc.sync.dma_start(out=s2_sb[:], in_=s2_ext[:])
            b2_sb = cpool.tile([CP, 1], f32)
            nc.sync.dma_start(out=b2_sb[:], in_=b2_ext[:])
            resw_sb = cpool.tile([C, 96], bf16)
            nc.sync.dma_start(out=resw_sb[:], in_=resw_ext[:])

            def emit_loads(n):
                # group0 input = concat channels 0..95  = [x[0:48], prev[48:96]]
                # group1 input = concat channels 96..191 = [x[48:96], prev[0:48]]
                # casting DMAs (f32 -> bf16 in flight) must go via gpsimd rings
                g0_raw = rawp.tile([C, PIX], bf16, tag="g0raw", name=f"g0_raw{n}")
                nc.gpsimd.dma_start(out=g0_raw[0:48, :], in_=x_ext[n, 0:48, :])
                nc.gpsimd.dma_start(out=g0_raw[48:96, :], in_=p_ext[n, 48:96, :])
                g1_raw = rawp.tile([C, PIX], bf16, tag="g1raw", name=f"g1_raw{n}")
                nc.gpsimd.dma_start(out=g1_raw[0:48, :], in_=x_ext[n, 48:96, :])
                nc.gpsimd.dma_start(out=g1_raw[48:96, :], in_=p_ext[n, 0:48, :])

                # contiguous bf16 copy of x for the single-matmul residual
                xres = outp.tile([C, PIX], bf16, tag="xres", name=f"xres{n}")
                nc.sync.dma_start(out=xres[0:48, :], in_=g0_raw[0:48, :])
                nc.sync.dma_start(out=xres[48:96, :], in_=g1_raw[0:48, :])

                # bn1 + relu (scale folded into w1): a = max(z + bias1, 0)
                g0_act = actp.tile([C, PIX], bf16, tag="g0act", name=f"g0_act{n}")
                nc.vector.tensor_scalar(
                    g0_act[:], g0_raw[:], bias1_sb[:, 0:1], 0.0,
                    mybir.AluOpType.add, mybir.AluOpType.max,
                )
                g1_act = actp.tile([C, PIX], bf16, tag="g1act", name=f"g1_act{n}")
                nc.vector.tensor_scalar(
                    g1_act[:], g1_raw[:], bias1_sb[:, 1:2], 0.0,
                    mybir.AluOpType.add, mybir.AluOpType.max,
                )
                return g0_raw, g1_raw, xres, g0_act, g1_act

            nxt = emit_loads(0)
            for n in range(N_PER):
                g0_raw, g1_raw, xres, g0_act, g1_act = nxt
                if n + 1 < N_PER:
                    nxt = emit_loads(n + 1)

                b_sb = bvp.tile([CP, PIX], bf16, tag="b")
                v_sb = bvp.tile([CP, PIX], bf16, tag="v")
                bm_sb = bvp.tile([CP, PIX], bf16, tag="bm")
                bp_sb = bvp.tile([CP, PIX], bf16, tag="bp")
                fmap_sb = outp.tile([CP, PIX], bf16, tag="fmap")
                out_sb = outp.tile([C, PIX], bf16, tag="out")

                # conv1 (groups=2) + bn2(relu) + fmap eviction, per 2-bank chunk
                for cth in range(NPAIR):
                    fp = fpsum.tile([CP, 2 * BANK], f32, tag="fp")
                    for k in range(2):
                        t = 2 * cth + k
                        sl = slice(t * TW, (t + 1) * TW)
                        pb = slice(k * BANK, k * BANK + TW)
                        nc.tensor.matmul(
                            fp[0:64, pb], w1_sb[:, 0:64],
                            g0_act[:, sl], start=True, stop=True,
                        )
                        nc.tensor.matmul(
                            fp[64:112, pb], w1_sb[:, 64:112],
                            g1_act[:, sl], start=True, stop=True,
                        )
                    fpv = fp.rearrange("p (b w) -> p b w", w=BANK)[:, :, 0:TW]
                    csl = slice(cth * 2 * TW, (cth + 1) * 2 * TW)
                    bv = b_sb[:, csl].rearrange("p (b w) -> p b w", w=TW)
                    nc.scalar.activation(
                        bv, fpv, mybir.ActivationFunctionType.Relu,
                        bias=b2_sb[:, 0:1], scale=s2_sb[:, 0:1],
                    )
                    fv = fmap_sb[:, csl].rearrange("p (b w) -> p b w", w=TW)
                    nc.scalar.activation(
                        fv, fpv, mybir.ActivationFunctionType.Copy,
                    )

                nc.sync.dma_start(out=fmap_ext[n, 0:48, :], in_=fmap_sb[0:48, :])
                nc.sync.dma_start(out=fmap_ext[n, 48:96, :], in_=fmap_sb[64:112, :])

                # row pass of the shift: v[c,i,:] = sum_oy wr[c,oy]*b[c,i+oy,:]
                # tensor_scalar (4x) + tensor_tensor (2x) only; no 1x STT ops.
                # Two halves, with the cross-half halo rows handled in the
                # second batch so every read refers to already-written data.
                HALF = PIX // 2
                for h0, h1 in ((0, HALF), (HALF, PIX)):
                    hs = slice(h0, h1)
                    nc.vector.tensor_scalar(
                        v_sb[:, hs], b_sb[:, hs], wr_sb[:, 1:2], None,
                        mybir.AluOpType.mult,
                    )
                    nc.vector.tensor_scalar(
                        bm_sb[:, hs], b_sb[:, hs], wr_sb[:, 0:1], None,
                        mybir.AluOpType.mult,
                    )
                    nc.vector.tensor_scalar(
                        bp_sb[:, hs], b_sb[:, hs], wr_sb[:, 2:3], None,
                        mybir.AluOpType.mult,
                    )
                    if h0 == 0:
                        # rows 1..27: bm rows 0..26 ; rows 0..26: bp rows 1..27
                        nc.vector.tensor_tensor(
                            v_sb[:, W:HALF], bm_sb[:, 0:HALF - W], v_sb[:, W:HALF],
                            mybir.AluOpType.add,
                        )
                        nc.vector.tensor_tensor(
                            v_sb[:, 0:HALF - W], bp_sb[:, W:HALF], v_sb[:, 0:HALF - W],
                            mybir.AluOpType.add,
                        )
                    else:
                        # rows 28..55: bm rows 27..54 ; rows 27..54: bp rows 28..55
                        nc.vector.tensor_tensor(
                            v_sb[:, HALF:PIX], bm_sb[:, HALF - W:PIX - W],
                            v_sb[:, HALF:PIX], mybir.AluOpType.add,
                        )
                        nc.vector.tensor_tensor(
                            v_sb[:, HALF - W:PIX - W], bp_sb[:, HALF:PIX],
                            v_sb[:, HALF - W:PIX - W], mybir.AluOpType.add,
                        )

                v3 = v_sb.rearrange("p (r w) -> p r w", w=W)

                # conv2 (col taps folded into weights) + residual, then evict
                for cth in range(NPAIR):
                    op = opsum.tile([C, 2 * BANK], f32, tag="op")
                    for k in range(2):
                        t = 2 * cth + k
                        sl = slice(t * TW, (t + 1) * TW)
                        pb = slice(k * BANK, k * BANK + TW)
                        r0 = t * RT
                        op3 = op[:, pb].rearrange("p (r w) -> p r w", w=W)
                        nc.tensor.matmul(
                            op[:, pb], w2_sb[:, 96:192], v_sb[:, sl],
                            start=True, stop=False, skip_group_check=True,
                        )
                        nc.tensor.matmul(
                            op3[:, :, 1:W], w2_sb[:, 0:96],
                            v3[:, r0:r0 + RT, 0:W - 1],
                            start=False, stop=False, skip_group_check=True,
                        )
                        nc.tensor.matmul(
                            op3[:, :, 0:W - 1], w2_sb[:, 192:288],
                            v3[:, r0:r0 + RT, 1:W],
                            start=False, stop=False, skip_group_check=True,
                        )
                        nc.tensor.matmul(
                            op[:, pb], resw_sb[:, 0:96], xres[:, sl],
                            start=False, stop=True, skip_group_check=True,
                        )
                    opv = op.rearrange("p (b w) -> p b w", w=BANK)[:, :, 0:TW]
                    csl = slice(cth * 2 * TW, (cth + 1) * 2 * TW)
                    ov = out_sb[:, csl].rearrange("p (b w) -> p b w", w=TW)
                    nc.scalar.activation(
                        ov, opv, mybir.ActivationFunctionType.Copy,
                    )

                nc.sync.dma_start(out=out_ext[n, :, :], in_=out_sb[:, :])

    nc.compile()
    return nc


def _prep_consts(bn1_gamma, bn1_beta, bn1_mean, bn1_var,
                 bn2_gamma, bn2_beta, bn2_mean, bn2_var, w1, w2, shift):
    s1 = bn1_gamma / np.sqrt(bn1_var + EPS)
    t1 = bn1_beta - bn1_mean * s1
    bias1 = (t1 / s1).astype(np.float32).reshape(2, C).T.copy()  # [96, 2]

    # padded index for original fmap channel c
    pidx = np.concatenate([np.arange(48), 64 + np.arange(48)])  # [96]

    s2f = bn2_gamma / np.sqrt(bn2_var + EPS)
    b2f = bn2_beta - bn2_mean * s2f
    s2 = np.zeros((CP, 1), np.float32)
    b2 = np.zeros((CP, 1), np.float32)
    s2[pidx, 0] = s2f
    b2[pidx, 0] = b2f

    w1m = w1[:, :, 0, 0]  # (96 out, 96 in-per-group)
    w1t = np.zeros((C, CP), np.float32)
    w1t[:, 0:48] = (w1m[0:48] * s1[None, 0:96]).T       # group0 lhsT [96K, 48M]
    w1t[:, 64:112] = (w1m[48:96] * s1[None, 96:192]).T  # group1 lhsT

    dy, dx = shift[:, 0].astype(np.float64), shift[:, 1].astype(np.float64)
    ay = np.floor(dy)
    ax = np.floor(dx)
    fy = dy - ay
    fx = dx - ax
    wrf = np.zeros((C, 3), np.float32)
    wcf = np.zeros((C, 3), np.float32)
    for c in range(C):
        iy = int(ay[c]) + 1   # -1 -> 0, 0 -> 1
        ix = int(ax[c]) + 1
        wrf[c, iy] += 1.0 - fy[c]
        wrf[c, iy + 1] += fy[c]
        wcf[c, ix] += 1.0 - fx[c]
        wcf[c, ix + 1] += fx[c]
    wr = np.zeros((CP, 3), np.float32)
    wr[pidx] = wrf

    w2m = w2[:, :, 0, 0]  # (96 out, 32 in-per-group)
    w2full = np.zeros((C, C), np.float32)
    for g in range(3):
        w2full[32 * g:32 * g + 32, 32 * g:32 * g + 32] = w2m[32 * g:32 * g + 32]
    w2x = np.zeros((CP, 288), np.float32)
    for k in range(3):
        # lhsT[pidx[c], o] = w2full[o, c] * wc[c, k]
        w2x[pidx, 96 * k:96 * k + 96] = w2full.T * wcf[:, k:k + 1]

    # residual: identity matmul from the contiguous xres tile
    resw = np.eye(C, dtype=np.float32)

    return {
        "bias1": bias1,
        "s2": s2,
        "b2": b2,
        "w1t": w1t.astype(ml_dtypes.bfloat16),
        "w2x": w2x.astype(ml_dtypes.bfloat16),
        "wr": wr,
        "resw": resw.astype(ml_dtypes.bfloat16),
    }


_NC_CACHE = {}


def kernel(x, prev_fmap, bn1_gamma, bn1_beta, bn1_mean, bn1_var,
           bn2_gamma, bn2_beta, bn2_mean, bn2_var, w1, w2, shift):
    global LAST_EXEC_NS
    x = np.ascontiguousarray(np.asarray(x, np.float32))
    prev_fmap = np.ascontiguousarray(np.asarray(prev_fmap, np.float32))
    consts = _prep_consts(
        np.asarray(bn1_gamma, np.float32), np.asarray(bn1_beta, np.float32),
        np.asarray(bn1_mean, np.float32), np.asarray(bn1_var, np.float32),
        np.asarray(bn2_gamma, np.float32), np.asarray(bn2_beta, np.float32),
        np.asarray(bn2_mean, np.float32), np.asarray(bn2_var, np.float32),
        np.asarray(w1, np.float32), np.asarray(w2, np.float32),
        np.asarray(shift, np.float32))

    if "nc" not in _NC_CACHE:
        _NC_CACHE["nc"] = _build_nc()
    nc = _NC_CACHE["nc"]

    NB = x.shape[0]
    xs = x.reshape(N_CORES, N_PER, C, PIX)
    ps = prev_fmap.reshape(N_CORES, N_PER, C, PIX)
    in_maps = [
        {"x": xs[i], "prev": ps[i], **consts}
        for i in range(N_CORES)
    ]

    trace = bool(os.environ.get("CC_KERNEL_TRACE"))
    res = run_bass_kernel_spmd(
        nc, in_maps, core_ids=list(range(N_CORES)), trace=trace,
    )
    LAST_EXEC_NS = res.exec_time_ns

    out = np.empty((NB, C, PIX), np.float32)
    fmap = np.empty((NB, C, PIX), np.float32)
    for i in range(N_CORES):
        out[i * N_PER:(i + 1) * N_PER] = res.results[i]["out"].astype(np.float32)
        fmap[i * N_PER:(i + 1) * N_PER] = res.results[i]["fmap"].astype(np.float32)
    return (out.reshape(NB, C, H, W), fmap.reshape(NB, C, H, W))


# revision 10
# speedup vs baseline: 5.5479x; 1.0093x over previous
"""Trainium2 Bass kernel for nn_BasicBlock (dense_cnn, active-shift block).

Data-parallel over batch: 32 images -> 4 per NeuronCore across 8 cores.
Per-core layout: channels on SBUF partitions, pixels (H*W) on the free dim.

Math restructure (validated vs the jax reference in fp32 to ~1e-7):
  - bn1+relu:  relu(s1*z + t1) = s1 * relu(z + t1/s1); the s1 scale is folded
    into the columns of w1, so bn1 is a single add+max tensor_scalar on
    VectorE (bf16, 4x mode).
  - conv1 (groups=2, bf16): two matmuls per pixel tile.  PE matmul outputs
    must start at partition 0 or 64, so the 96 fmap channels live interleaved
    on partitions [0:48] and [64:112]; partitions [48:64] are written zero via
    zero weight columns.  Everything after conv1 uses this padded
    112-partition layout (elementwise ops cost by free dim only, so the dead
    partitions are free); the fmap DMA and conv2 weights fold it back.
  - bn2+relu: ScalarE activation (per-partition scale/bias) from PSUM -> bf16.
  - active_shift is separable bilinear: a row pass on VectorE
    (v = wr0*b; bm = wrm*b; bp = wrp*b; v += shift(bm); v += shift(bp) --
    tensor_scalar 4x + tensor_tensor 2x only, no 1x-mode ops) and a column
    pass folded into conv2's weights (3 matmuls with column-shifted APs).
  - conv2 (groups=3) is a block-diagonal matmul over the padded layout; the
    +x residual is accumulated in PSUM via two shifted-identity matmuls from
    the bf16 raw tiles; ScalarE evicts the result.

dtype strategy: inputs are cast f32->bf16 by the load DMAs (GpSimd-initiated
casting DMAs; the GpSimd ALU pipeline stays empty -- its tensor ops are both
slow and poison concurrent VectorE ops via SBUF port sharing).  Outputs are
produced as bf16, DMA'd as bf16 (halves output HBM traffic) and widened to
f32 on the host.  End-to-end absmax-relative error ~3e-3.

Spatial tiling: 7 rows (392 px) per PSUM bank; pairs of banks share one PSUM
tile so bn2 / copies run at 784-px granularity (amortizes per-op overheads).
"""

import os
import numpy as np
import ml_dtypes

import concourse.bass as bass
import concourse.bacc as bacc
import concourse.mybir as mybir
from concourse import tile
from concourse.bass_utils import run_bass_kernel_spmd

EPS = 1e-5
N_CORES = 8
N_PER = 4            # images per core
C = 96
CP = 112             # padded channel count for the post-conv1 layout
H = 56
W = 56
PIX = H * W          # 3136
RT = 7               # rows per spatial tile
TW = RT * W          # 392 pixels per tile (one PSUM bank each)
NT = H // RT         # 8 tiles per image
NPAIR = NT // 2      # 4 two-bank chunks per image
BANK = 512           # fp32 elems per PSUM bank

f32 = mybir.dt.float32
bf16 = mybir.dt.bfloat16

LAST_EXEC_NS = None


def _build_nc():
    nc = bacc.Bacc("TRN2", target_bir_lowering=False, debug=False, num_swdge_queues=4)

    x_ext = nc.declare_dram_parameter("x", [N_PER, C, PIX], f32, isOutput=False)
    p_ext = nc.declare_dram_parameter("prev", [N_PER, C, PIX], f32, isOutput=False)
    bias1_ext = nc.declare_dram_parameter("bias1", [C, 2], f32, isOutput=False)
    s2_ext = nc.declare_dram_parameter("s2", [CP, 1], f32, isOutput=False)
    b2_ext = nc.declare_dram_parameter("b2", [CP, 1], f32, isOutput=False)
    w1t_ext = nc.declare_dram_parameter("w1t", [C, CP], bf16, isOutput=False)
    w2x_ext = nc.declare_dram_parameter("w2x", [CP, 288], bf16, isOutput=False)
    wr_ext = nc.declare_dram_parameter("wr", [CP, 3], f32, isOutput=False)
    resw_ext = nc.declare_dram_parameter("resw", [C, 96], bf16, isOutput=False)
    out_ext = nc.declare_dram_parameter("out", [N_PER, C, PIX], bf16, isOutput=True)
    fmap_ext = nc.declare_dram_parameter("fmap", [N_PER, C, PIX], bf16, isOutput=True)

    with tile.TileContext(nc) as tc:
        with (
            tc.tile_pool(name="consts", bufs=1) as cpool,
            tc.tile_pool(name="raw", bufs=2) as rawp,
            tc.tile_pool(name="act", bufs=2) as actp,
            tc.tile_pool(name="bv", bufs=2) as bvp,
            tc.tile_pool(name="outs", bufs=2) as outp,
            tc.tile_pool(name="fpsum", bufs=2, space="PSUM") as fpsum,
            tc.tile_pool(name="opsum", bufs=2, space="PSUM") as opsum,
        ):
            w1_sb = cpool.tile([C, CP], bf16)
            nc.sync.dma_start(out=w1_sb[:], in_=w1t_ext[:])
            w2_sb = cpool.tile([CP, 288], bf16)
            nc.sync.dma_start(out=w2_sb[:], in_=w2x_ext[:])
            wr_sb = cpool.tile([CP, 3], f32)
            nc.sync.dma_start(out=wr_sb[:], in_=wr_ext[:])
            bias1_sb = cpool.tile([C, 2], f32)
            nc.sync.dma_start(out=bias1_sb[:], in_=bias1_ext[:])
            s2_sb = cpool.tile([CP, 1], f32)
            nc.sync.dma_start(out=s2_sb[:], in_=s2_ext[:])
            b2_sb = cpool.tile([CP, 1], f32)
            nc.sync.dma_start(out=b2_sb[:], in_=b2_ext[:])
            resw_sb = cpool.tile([C, 96], bf16)
            nc.sync.dma_start(out=resw_sb[:], in_=resw_ext[:])

            def emit_loads(n):
                # group0 input = concat channels 0..95  = [x[0:48], prev[48:96]]
                # group1 input = concat channels 96..191 = [x[48:96], prev[0:48]]
                # casting DMAs (f32 -> bf16 in flight) must go via gpsimd rings
                g0_raw = rawp.tile([C, PIX], bf16, tag="g0raw", name=f"g0_raw{n}")
                nc.gpsimd.dma_start(out=g0_raw[0:48, :], in_=x_ext[n, 0:48, :])
                nc.gpsimd.dma_start(out=g0_raw[48:96, :], in_=p_ext[n, 48:96, :])
                g1_raw = rawp.tile([C, PIX], bf16, tag="g1raw", name=f"g1_raw{n}")
                nc.gpsimd.dma_start(out=g1_raw[0:48, :], in_=x_ext[n, 48:96, :])
                nc.gpsimd.dma_start(out=g1_raw[48:96, :], in_=p_ext[n, 0:48, :])

                # contiguous bf16 copy of x for the single-matmul residual
                xres = outp.tile([C, PIX], bf16, tag="xres", name=f"xres{n}")
                nc.sync.dma_start(out=xres[0:48, :], in_=g0_raw[0:48, :])
                nc.sync.dma_start(out=xres[48:96, :], in_=g1_raw[0:48, :])

                # bn1 + relu (scale folded into w1): a = max(z + bias1, 0)
                g0_act = actp.tile([C, PIX], bf16, tag="g0act", name=f"g0_act{n}")
                nc.vector.tensor_scalar(
                    g0_act[:], g0_raw[:], bias1_sb[:, 0:1], 0.0,
                    mybir.AluOpType.add, mybir.AluOpType.max,
                )
                g1_act = actp.tile([C, PIX], bf16, tag="g1act", name=f"g1_act{n}")
                nc.vector.tensor_scalar(
                    g1_act[:], g1_raw[:], bias1_sb[:, 1:2], 0.0,
                    mybir.AluOpType.add, mybir.AluOpType.max,
                )
                return g0_raw, g1_raw, xres, g0_act, g1_act

            nxt = emit_loads(0)
            for n in range(N_PER):
                g0_raw, g1_raw, xres, g0_act, g1_act = nxt
                if n + 1 < N_PER:
                    nxt = emit_loads(n + 1)

                b_sb = bvp.tile([CP, PIX], bf16, tag="b")
                v_sb = bvp.tile([CP, PIX], bf16, tag="v")
                bm_sb = bvp.tile([CP, PIX], bf16, tag="bm")
                bp_sb = bvp.tile([CP, PIX], bf16, tag="bp")
                fmap_sb = outp.tile([CP, PIX], bf16, tag="fmap")
                out_sb = outp.tile([C, PIX], bf16, tag="out")

                # conv1 (groups=2) + bn2(relu) + fmap eviction, per 2-bank chunk
                for cth in range(NPAIR):
                    fp = fpsum.tile([CP, 2 * BANK], f32, tag="fp")
                    for k in range(2):
                        t = 2 * cth + k
                        sl = slice(t * TW, (t + 1) * TW)
                        pb = slice(k * BANK, k * BANK + TW)
                        nc.tensor.matmul(
                            fp[0:64, pb], w1_sb[:, 0:64],
                            g0_act[:, sl], start=True, stop=True,
                        )
                        nc.tensor.matmul(
                            fp[64:112, pb], w1_sb[:, 64:112],
                            g1_act[:, sl], start=True, stop=True,
                        )
                    fpv = fp.rearrange("p (b w) -> p b w", w=BANK)[:, :, 0:TW]
                    csl = slice(cth * 2 * TW, (cth + 1) * 2 * TW)
                    bv = b_sb[:, csl].rearrange("p (b w) -> p b w", w=TW)
                    nc.scalar.activation(
                        bv, fpv, mybir.ActivationFunctionType.Relu,
                        bias=b2_sb[:, 0:1], scale=s2_sb[:, 0:1],
                    )
                    fv = fmap_sb[:, csl].rearrange("p (b w) -> p b w", w=TW)
                    nc.scalar.activation(
                        fv, fpv, mybir.ActivationFunctionType.Copy,
                    )
                    if cth % 2 == 1:
                        hsl = slice((cth - 1) * 2 * TW, (cth + 1) * 2 * TW)
                        nc.sync.dma_start(out=fmap_ext[n, 0:48, hsl],
                                          in_=fmap_sb[0:48, hsl])
                        nc.sync.dma_start(out=fmap_ext[n, 48:96, hsl],
                                          in_=fmap_sb[64:112, hsl])

                # row pass of the shift: v[c,i,:] = sum_oy wr[c,oy]*b[c,i+oy,:]
                # tensor_scalar (4x) + tensor_tensor (2x) only; no 1x STT ops.
                # Two halves, with the cross-half halo rows handled in the
                # second batch so every read refers to already-written data.
                HALF = PIX // 2
                for h0, h1 in ((0, HALF), (HALF, PIX)):
                    hs = slice(h0, h1)
                    nc.vector.tensor_scalar(
                        v_sb[:, hs], b_sb[:, hs], wr_sb[:, 1:2], None,
                        mybir.AluOpType.mult,
                    )
                    nc.vector.tensor_scalar(
                        bm_sb[:, hs], b_sb[:, hs], wr_sb[:, 0:1], None,
                        mybir.AluOpType.mult,
                    )
                    nc.vector.tensor_scalar(
                        bp_sb[:, hs], b_sb[:, hs], wr_sb[:, 2:3], None,
                        mybir.AluOpType.mult,
                    )
                    if h0 == 0:
                        # rows 1..27: bm rows 0..26 ; rows 0..26: bp rows 1..27
                        nc.vector.tensor_tensor(
                            v_sb[:, W:HALF], bm_sb[:, 0:HALF - W], v_sb[:, W:HALF],
                            mybir.AluOpType.add,
                        )
                        nc.vector.tensor_tensor(
                            v_sb[:, 0:HALF - W], bp_sb[:, W:HALF], v_sb[:, 0:HALF - W],
                            mybir.AluOpType.add,
                        )
                    else:
                        # rows 28..55: bm rows 27..54 ; rows 27..54: bp rows 28..55
                        nc.vector.tensor_tensor(
                            v_sb[:, HALF:PIX], bm_sb[:, HALF - W:PIX - W],
                            v_sb[:, HALF:PIX], mybir.AluOpType.add,
                        )
                        nc.vector.tensor_tensor(
                            v_sb[:, HALF - W:PIX - W], bp_sb[:, HALF:PIX],
                            v_sb[:, HALF - W:PIX - W], mybir.AluOpType.add,
                        )

                v3 = v_sb.rearrange("p (r w) -> p r w", w=W)

                # conv2 (col taps folded into weights) + residual, then evict
                for cth in range(NPAIR):
                    op = opsum.tile([C, 2 * BANK], f32, tag="op")
                    for k in range(2):
                        t = 2 * cth + k
                        sl = slice(t * TW, (t + 1) * TW)
                        pb = slice(k * BANK, k * BANK + TW)
                        r0 = t * RT
                        op3 = op[:, pb].rearrange("p (r w) -> p r w", w=W)
                        nc.tensor.matmul(
                            op[:, pb], w2_sb[:, 96:192], v_sb[:, sl],
                            start=True, stop=False, skip_group_check=True,
                        )
                        nc.tensor.matmul(
                            op3[:, :, 1:W], w2_sb[:, 0:96],
                            v3[:, r0:r0 + RT, 0:W - 1],
                            start=False, stop=False, skip_group_check=True,
                        )
                        nc.tensor.matmul(
                            op3[:, :, 0:W - 1], w2_sb[:, 192:288],
                            v3[:, r0:r0 + RT, 1:W],
                            start=False, stop=False, skip_group_check=True,
                        )
                        nc.tensor.matmul(
                            op[:, pb], resw_sb[:, 0:96], xres[:, sl],
                            start=False, stop=True, skip_group_check=True,
                        )
                    opv = op.rearrange("p (b w) -> p b w", w=BANK)[:, :, 0:TW]
                    csl = slice(cth * 2 * TW, (cth + 1) * 2 * TW)
                    ov = out_sb[:, csl].rearrange("p (b w) -> p b w", w=TW)
                    nc.scalar.activation(
                        ov, opv, mybir.ActivationFunctionType.Copy,
                    )
                    if cth % 2 == 1:
                        hsl = slice((cth - 1) * 2 * TW, (cth + 1) * 2 * TW)
                        nc.sync.dma_start(out=out_ext[n, :, hsl],
                                          in_=out_sb[:, hsl])

    nc.compile()
    return nc


def _prep_consts(bn1_gamma, bn1_beta, bn1_mean, bn1_var,
                 bn2_gamma, bn2_beta, bn2_mean, bn2_var, w1, w2, shift):
    s1 = bn1_gamma / np.sqrt(bn1_var + EPS)
    t1 = bn1_beta - bn1_mean * s1
    bias1 = (t1 / s1).astype(np.float32).reshape(2, C).T.copy()  # [96, 2]

    # padded index for original fmap channel c
    pidx = np.concatenate([np.arange(48), 64 + np.arange(48)])  # [96]

    s2f = bn2_gamma / np.sqrt(bn2_var + EPS)
    b2f = bn2_beta - bn2_mean * s2f
    s2 = np.zeros((CP, 1), np.float32)
    b2 = np.zeros((CP, 1), np.float32)
    s2[pidx, 0] = s2f
    b2[pidx, 0] = b2f

    w1m = w1[:, :, 0, 0]  # (96 out, 96 in-per-group)
    w1t = np.zeros((C, CP), np.float32)
    w1t[:, 0:48] = (w1m[0:48] * s1[None, 0:96]).T       # group0 lhsT [96K, 48M]
    w1t[:, 64:112] = (w1m[48:96] * s1[None, 96:192]).T  # group1 lhsT

    dy, dx = shift[:, 0].astype(np.float64), shift[:, 1].astype(np.float64)
    ay = np.floor(dy)
    ax = np.floor(dx)
    fy = dy - ay
    fx = dx - ax
    wrf = np.zeros((C, 3), np.float32)
    wcf = np.zeros((C, 3), np.float32)
    for c in range(C):
        iy = int(ay[c]) + 1   # -1 -> 0, 0 -> 1
        ix = int(ax[c]) + 1
        wrf[c, iy] += 1.0 - fy[c]
        wrf[c, iy + 1] += fy[c]
        wcf[c, ix] += 1.0 - fx[c]
        wcf[c, ix + 1] += fx[c]
    wr = np.zeros((CP, 3), np.float32)
    wr[pidx] = wrf

    w2m = w2[:, :, 0, 0]  # (96 out, 32 in-per-group)
    w2full = np.zeros((C, C), np.float32)
    for g in range(3):
        w2full[32 * g:32 * g + 32, 32 * g:32 * g + 32] = w2m[32 * g:32 * g + 32]
    w2x = np.zeros((CP, 288), np.float32)
    for k in range(3):
        # lhsT[pidx[c], o] = w2full[o, c] * wc[c, k]
        w2x[pidx, 96 * k:96 * k + 96] = w2full.T * wcf[:, k:k + 1]

    # residual: identity matmul from the contiguous xres tile
    resw = np.eye(C, dtype=np.float32)

    return {
        "bias1": bias1,
        "s2": s2,
        "b2": b2,
        "w1t": w1t.astype(ml_dtypes.bfloat16),
        "w2x": w2x.astype(ml_dtypes.bfloat16),
        "wr": wr,
        "resw": resw.astype(ml_dtypes.bfloat16),
    }


_NC_CACHE = {}


def kernel(x, prev_fmap, bn1_gamma, bn1_beta, bn1_mean, bn1_var,
           bn2_gamma, bn2_beta, bn2_mean, bn2_var, w1, w2, shift):
    global LAST_EXEC_NS
    x = np.ascontiguousarray(np.asarray(x, np.float32))
    prev_fmap = np.ascontiguousarray(np.asarray(prev_fmap, np.float32))
    consts = _prep_consts(
        np.asarray(bn1_gamma, np.float32), np.asarray(bn1_beta, np.float32),
        np.asarray(bn1_mean, np.float32), np.asarray(bn1_var, np.float32),
        np.asarray(bn2_gamma, np.float32), np.asarray(bn2_beta, np.float32),
        np.asarray(bn2_mean, np.float32), np.asarray(bn2_var, np.float32),
        np.asarray(w1, np.float32), np.asarray(w2, np.float32),
        np.asarray(shift, np.float32))

    if "nc" not in _NC_CACHE:
        _NC_CACHE["nc"] = _build_nc()
    nc = _NC_CACHE["nc"]

    NB = x.shape[0]
    xs = x.reshape(N_CORES, N_PER, C, PIX)
    ps = prev_fmap.reshape(N_CORES, N_PER, C, PIX)
    in_maps = [
        {"x": xs[i], "prev": ps[i], **consts}
        for i in range(N_CORES)
    ]

    trace = bool(os.environ.get("CC_KERNEL_TRACE"))
    res = run_bass_kernel_spmd(
        nc, in_maps, core_ids=list(range(N_CORES)), trace=trace,
    )
    LAST_EXEC_NS = res.exec_time_ns

    out = np.empty((NB, C, PIX), np.float32)
    fmap = np.empty((NB, C, PIX), np.float32)
    for i in range(N_CORES):
        out[i * N_PER:(i + 1) * N_PER] = res.results[i]["out"].astype(np.float32)
        fmap[i * N_PER:(i + 1) * N_PER] = res.results[i]["fmap"].astype(np.float32)
    return (out.reshape(NB, C, H, W), fmap.reshape(NB, C, H, W))


# revision 11
# speedup vs baseline: 5.7836x; 1.0425x over previous
"""Trainium2 Bass kernel for nn_BasicBlock (dense_cnn, active-shift block).

Data-parallel over batch: 32 images -> 4 per NeuronCore across 8 cores.
Per-core layout: channels on SBUF partitions, pixels (H*W) on the free dim.

Math restructure (validated vs the jax reference in fp32 to ~1e-7):
  - bn1+relu:  relu(s1*z + t1) = s1 * relu(z + t1/s1); the s1 scale is folded
    into the columns of w1, so bn1 is a single add+max tensor_scalar on
    VectorE (bf16, 4x mode).
  - conv1 (groups=2, bf16): two matmuls per pixel tile.  PE matmul outputs
    must start at partition 0 or 64, so the 96 fmap channels live interleaved
    on partitions [0:48] and [64:112]; partitions [48:64] are written zero via
    zero weight columns.  Everything after conv1 uses this padded
    112-partition layout (elementwise ops cost by free dim only, so the dead
    partitions are free); the fmap DMA and conv2 weights fold it back.
  - bn2+relu: ScalarE activation (per-partition scale/bias) from PSUM -> bf16.
  - active_shift is separable bilinear: a row pass on VectorE
    (v = wr0*b; bm = wrm*b; bp = wrp*b; v += shift(bm); v += shift(bp) --
    tensor_scalar 4x + tensor_tensor 2x only, no 1x-mode ops) and a column
    pass folded into conv2's weights (3 matmuls with column-shifted APs).
  - conv2 (groups=3) is a block-diagonal matmul over the padded layout; the
    +x residual is accumulated in PSUM via two shifted-identity matmuls from
    the bf16 raw tiles; ScalarE evicts the result.

dtype strategy: inputs are cast f32->bf16 by the load DMAs (GpSimd-initiated
casting DMAs; the GpSimd ALU pipeline stays empty -- its tensor ops are both
slow and poison concurrent VectorE ops via SBUF port sharing).  Outputs are
produced as bf16, DMA'd as bf16 (halves output HBM traffic) and widened to
f32 on the host.  End-to-end absmax-relative error ~3e-3.

Spatial tiling: 7 rows (392 px) per PSUM bank; pairs of banks share one PSUM
tile so bn2 / copies run at 784-px granularity (amortizes per-op overheads).
"""

import os
import numpy as np
import ml_dtypes

import concourse.bass as bass
import concourse.bacc as bacc
import concourse.mybir as mybir
from concourse import tile
from concourse.bass_utils import run_bass_kernel_spmd

EPS = 1e-5
N_CORES = 8
N_PER = 4            # images per core
C = 96
CP = 112             # padded channel count for the post-conv1 layout
H = 56
W = 56
PIX = H * W          # 3136
RT = 7               # rows per spatial tile
TW = RT * W          # 392 pixels per tile (one PSUM bank each)
NT = H // RT         # 8 tiles per image
NPAIR = NT // 2      # 4 two-bank chunks per image
BANK = 512           # fp32 elems per PSUM bank

f32 = mybir.dt.float32
bf16 = mybir.dt.bfloat16

LAST_EXEC_NS = None


def _build_nc():
    nc = bacc.Bacc("TRN2", target_bir_lowering=False, debug=False, num_swdge_queues=4)

    x_ext = nc.declare_dram_parameter("x", [N_PER, C, PIX], f32, isOutput=False)
    p_ext = nc.declare_dram_parameter("prev", [N_PER, C, PIX], f32, isOutput=False)
    bias1_ext = nc.declare_dram_parameter("bias1", [C, 2], f32, isOutput=False)
    t2_ext = nc.declare_dram_parameter("t2", [CP, 1], f32, isOutput=False)
    w1t_ext = nc.declare_dram_parameter("w1t", [C, CP], bf16, isOutput=False)
    w2x_ext = nc.declare_dram_parameter("w2x", [CP, 288], bf16, isOutput=False)
    wr_ext = nc.declare_dram_parameter("wr", [CP, 3], f32, isOutput=False)
    resw_ext = nc.declare_dram_parameter("resw", [C, 96], bf16, isOutput=False)
    out_ext = nc.declare_dram_parameter("out", [N_PER, C, PIX], bf16, isOutput=True)
    fmap_ext = nc.declare_dram_parameter("fmap", [N_PER, C, PIX], bf16, isOutput=True)

    with tile.TileContext(nc) as tc:
        with (
            tc.tile_pool(name="consts", bufs=1) as cpool,
            tc.tile_pool(name="raw", bufs=2) as rawp,
            tc.tile_pool(name="act", bufs=2) as actp,
            tc.tile_pool(name="bv", bufs=2) as bvp,
            tc.tile_pool(name="outs", bufs=2) as outp,
            tc.tile_pool(name="fpsum", bufs=2, space="PSUM") as fpsum,
            tc.tile_pool(name="opsum", bufs=2, space="PSUM") as opsum,
        ):
            w1_sb = cpool.tile([C, CP], bf16)
            nc.sync.dma_start(out=w1_sb[:], in_=w1t_ext[:])
            w2_sb = cpool.tile([CP, 288], bf16)
            nc.sync.dma_start(out=w2_sb[:], in_=w2x_ext[:])
            wr_sb = cpool.tile([CP, 3], f32)
            nc.sync.dma_start(out=wr_sb[:], in_=wr_ext[:])
            bias1_sb = cpool.tile([C, 2], f32)
            nc.sync.dma_start(out=bias1_sb[:], in_=bias1_ext[:])
            t2_sb = cpool.tile([CP, 1], f32)
            nc.sync.dma_start(out=t2_sb[:], in_=t2_ext[:])
            resw_sb = cpool.tile([C, 96], bf16)
            nc.sync.dma_start(out=resw_sb[:], in_=resw_ext[:])

            def emit_loads(n):
                # group0 input = concat channels 0..95  = [x[0:48], prev[48:96]]
                # group1 input = concat channels 96..191 = [x[48:96], prev[0:48]]
                # casting DMAs (f32 -> bf16 in flight) must go via gpsimd rings
                g0_raw = rawp.tile([C, PIX], bf16, tag="g0raw", name=f"g0_raw{n}")
                nc.gpsimd.dma_start(out=g0_raw[0:48, :], in_=x_ext[n, 0:48, :])
                nc.gpsimd.dma_start(out=g0_raw[48:96, :], in_=p_ext[n, 48:96, :])
                g1_raw = rawp.tile([C, PIX], bf16, tag="g1raw", name=f"g1_raw{n}")
                nc.gpsimd.dma_start(out=g1_raw[0:48, :], in_=x_ext[n, 48:96, :])
                nc.gpsimd.dma_start(out=g1_raw[48:96, :], in_=p_ext[n, 0:48, :])

                # contiguous bf16 copy of x for the single-matmul residual
                xres = outp.tile([C, PIX], bf16, tag="xres", name=f"xres{n}")
                nc.sync.dma_start(out=xres[0:48, :], in_=g0_raw[0:48, :])
                nc.sync.dma_start(out=xres[48:96, :], in_=g1_raw[0:48, :])

                # bn1 + relu (scale folded into w1): a = max(z + bias1, 0)
                g0_act = actp.tile([C, PIX], bf16, tag="g0act", name=f"g0_act{n}")
                nc.vector.tensor_scalar(
                    g0_act[:], g0_raw[:], bias1_sb[:, 0:1], 0.0,
                    mybir.AluOpType.add, mybir.AluOpType.max,
                )
                g1_act = actp.tile([C, PIX], bf16, tag="g1act", name=f"g1_act{n}")
                nc.vector.tensor_scalar(
                    g1_act[:], g1_raw[:], bias1_sb[:, 1:2], 0.0,
                    mybir.AluOpType.add, mybir.AluOpType.max,
                )
                return g0_raw, g1_raw, xres, g0_act, g1_act

            nxt = emit_loads(0)
            for n in range(N_PER):
                g0_raw, g1_raw, xres, g0_act, g1_act = nxt
                if n + 1 < N_PER:
                    nxt = emit_loads(n + 1)

                b_sb = bvp.tile([CP, PIX], bf16, tag="b")
                v_sb = bvp.tile([CP, PIX], bf16, tag="v")
                bm_sb = bvp.tile([CP, PIX], bf16, tag="bm")
                bp_sb = bvp.tile([CP, PIX], bf16, tag="bp")
                fmap_sb = outp.tile([CP, PIX], bf16, tag="fmap")
                out_sb = outp.tile([C, PIX], bf16, tag="out")

                # conv1 (groups=2) + bn2(relu) + fmap eviction, per 2-bank chunk
                for cth in range(NPAIR):
                    fp = fpsum.tile([CP, 2 * BANK], f32, tag="fp")
                    for k in range(2):
                        t = 2 * cth + k
                        sl = slice(t * TW, (t + 1) * TW)
                        pb = slice(k * BANK, k * BANK + TW)
                        nc.tensor.matmul(
                            fp[0:64, pb], w1_sb[:, 0:64],
                            g0_act[:, sl], start=True, stop=True,
                        )
                        nc.tensor.matmul(
                            fp[64:112, pb], w1_sb[:, 64:112],
                            g1_act[:, sl], start=True, stop=True,
                        )
                    fpv = fp.rearrange("p (b w) -> p b w", w=BANK)[:, :, 0:TW]
                    csl = slice(cth * 2 * TW, (cth + 1) * 2 * TW)
                    fv = fmap_sb[:, csl].rearrange("p (b w) -> p b w", w=TW)
                    nc.scalar.activation(
                        fv, fpv, mybir.ActivationFunctionType.Copy,
                    )
                    if cth % 2 == 1:
                        hsl = slice((cth - 1) * 2 * TW, (cth + 1) * 2 * TW)
                        nc.sync.dma_start(out=fmap_ext[n, 0:48, hsl],
                                          in_=fmap_sb[0:48, hsl])
                        nc.sync.dma_start(out=fmap_ext[n, 48:96, hsl],
                                          in_=fmap_sb[64:112, hsl])

                # row pass of the shift: v[c,i,:] = sum_oy wr[c,oy]*b[c,i+oy,:]
                # tensor_scalar (4x) + tensor_tensor (2x) only; no 1x STT ops.
                # Two halves, with the cross-half halo rows handled in the
                # second batch so every read refers to already-written data.
                HALF = PIX // 2
                for h0, h1 in ((0, HALF), (HALF, PIX)):
                    hs = slice(h0, h1)
                    # bn2 (scale folded into wr): b' = max(fmap + b2/s2, 0)
                    nc.vector.tensor_scalar(
                        b_sb[:, hs], fmap_sb[:, hs], t2_sb[:, 0:1], 0.0,
                        mybir.AluOpType.add, mybir.AluOpType.max,
                    )
                    nc.vector.tensor_scalar(
                        v_sb[:, hs], b_sb[:, hs], wr_sb[:, 1:2], None,
                        mybir.AluOpType.mult,
                    )
                    nc.vector.tensor_scalar(
                        bm_sb[:, hs], b_sb[:, hs], wr_sb[:, 0:1], None,
                        mybir.AluOpType.mult,
                    )
                    nc.vector.tensor_scalar(
                        bp_sb[:, hs], b_sb[:, hs], wr_sb[:, 2:3], None,
                        mybir.AluOpType.mult,
                    )
                    if h0 == 0:
                        # rows 1..27: bm rows 0..26 ; rows 0..26: bp rows 1..27
                        nc.vector.tensor_tensor(
                            v_sb[:, W:HALF], bm_sb[:, 0:HALF - W], v_sb[:, W:HALF],
                            mybir.AluOpType.add,
                        )
                        nc.vector.tensor_tensor(
                            v_sb[:, 0:HALF - W], bp_sb[:, W:HALF], v_sb[:, 0:HALF - W],
                            mybir.AluOpType.add,
                        )
                    else:
                        # rows 28..55: bm rows 27..54 ; rows 27..54: bp rows 28..55
                        nc.vector.tensor_tensor(
                            v_sb[:, HALF:PIX], bm_sb[:, HALF - W:PIX - W],
                            v_sb[:, HALF:PIX], mybir.AluOpType.add,
                        )
                        nc.vector.tensor_tensor(
                            v_sb[:, HALF - W:PIX - W], bp_sb[:, HALF:PIX],
                            v_sb[:, HALF - W:PIX - W], mybir.AluOpType.add,
                        )

                v3 = v_sb.rearrange("p (r w) -> p r w", w=W)

                # conv2 (col taps folded into weights) + residual, then evict
                for cth in range(NPAIR):
                    op = opsum.tile([C, 2 * BANK], f32, tag="op")
                    for k in range(2):
                        t = 2 * cth + k
                        sl = slice(t * TW, (t + 1) * TW)
                        pb = slice(k * BANK, k * BANK + TW)
                        r0 = t * RT
                        op3 = op[:, pb].rearrange("p (r w) -> p r w", w=W)
                        nc.tensor.matmul(
                            op[:, pb], w2_sb[:, 96:192], v_sb[:, sl],
                            start=True, stop=False, skip_group_check=True,
                        )
                        nc.tensor.matmul(
                            op3[:, :, 1:W], w2_sb[:, 0:96],
                            v3[:, r0:r0 + RT, 0:W - 1],
                            start=False, stop=False, skip_group_check=True,
                        )
                        nc.tensor.matmul(
                            op3[:, :, 0:W - 1], w2_sb[:, 192:288],
                            v3[:, r0:r0 + RT, 1:W],
                            start=False, stop=False, skip_group_check=True,
                        )
                        nc.tensor.matmul(
                            op[:, pb], resw_sb[:, 0:96], xres[:, sl],
                            start=False, stop=True, skip_group_check=True,
                        )
                    opv = op.rearrange("p (b w) -> p b w", w=BANK)[:, :, 0:TW]
                    csl = slice(cth * 2 * TW, (cth + 1) * 2 * TW)
                    ov = out_sb[:, csl].rearrange("p (b w) -> p b w", w=TW)
                    nc.scalar.activation(
                        ov, opv, mybir.ActivationFunctionType.Copy,
                    )
                    if cth % 2 == 1:
                        hsl = slice((cth - 1) * 2 * TW, (cth + 1) * 2 * TW)
                        nc.sync.dma_start(out=out_ext[n, :, hsl],
                                          in_=out_sb[:, hsl])

    nc.compile()
    return nc


def _prep_consts(bn1_gamma, bn1_beta, bn1_mean, bn1_var,
                 bn2_gamma, bn2_beta, bn2_mean, bn2_var, w1, w2, shift):
    s1 = bn1_gamma / np.sqrt(bn1_var + EPS)
    t1 = bn1_beta - bn1_mean * s1
    bias1 = (t1 / s1).astype(np.float32).reshape(2, C).T.copy()  # [96, 2]

    # padded index for original fmap channel c
    pidx = np.concatenate([np.arange(48), 64 + np.arange(48)])  # [96]

    s2f = bn2_gamma / np.sqrt(bn2_var + EPS)
    b2f = bn2_beta - bn2_mean * s2f
    t2 = np.zeros((CP, 1), np.float32)
    t2[pidx, 0] = b2f / s2f

    w1m = w1[:, :, 0, 0]  # (96 out, 96 in-per-group)
    w1t = np.zeros((C, CP), np.float32)
    w1t[:, 0:48] = (w1m[0:48] * s1[None, 0:96]).T       # group0 lhsT [96K, 48M]
    w1t[:, 64:112] = (w1m[48:96] * s1[None, 96:192]).T  # group1 lhsT

    dy, dx = shift[:, 0].astype(np.float64), shift[:, 1].astype(np.float64)
    ay = np.floor(dy)
    ax = np.floor(dx)
    fy = dy - ay
    fx = dx - ax
    wrf = np.zeros((C, 3), np.float32)
    wcf = np.zeros((C, 3), np.float32)
    for c in range(C):
        iy = int(ay[c]) + 1   # -1 -> 0, 0 -> 1
        ix = int(ax[c]) + 1
        wrf[c, iy] += 1.0 - fy[c]
        wrf[c, iy + 1] += fy[c]
        wcf[c, ix] += 1.0 - fx[c]
        wcf[c, ix + 1] += fx[c]
    wr = np.zeros((CP, 3), np.float32)
    wr[pidx] = wrf * s2f[:, None]

    w2m = w2[:, :, 0, 0]  # (96 out, 32 in-per-group)
    w2full = np.zeros((C, C), np.float32)
    for g in range(3):
        w2full[32 * g:32 * g + 32, 32 * g:32 * g + 32] = w2m[32 * g:32 * g + 32]
    w2x = np.zeros((CP, 288), np.float32)
    for k in range(3):
        # lhsT[pidx[c], o] = w2full[o, c] * wc[c, k]
        w2x[pidx, 96 * k:96 * k + 96] = w2full.T * wcf[:, k:k + 1]

    # residual: identity matmul from the contiguous xres tile
    resw = np.eye(C, dtype=np.float32)

    return {
        "bias1": bias1,
        "t2": t2,
        "w1t": w1t.astype(ml_dtypes.bfloat16),
        "w2x": w2x.astype(ml_dtypes.bfloat16),
        "wr": wr,
        "resw": resw.astype(ml_dtypes.bfloat16),
    }


_NC_CACHE = {}


def kernel(x, prev_fmap, bn1_gamma, bn1_beta, bn1_mean, bn1_var,
           bn2_gamma, bn2_beta, bn2_mean, bn2_var, w1, w2, shift):
    global LAST_EXEC_NS
    x = np.ascontiguousarray(np.asarray(x, np.float32))
    prev_fmap = np.ascontiguousarray(np.asarray(prev_fmap, np.float32))
    consts = _prep_consts(
        np.asarray(bn1_gamma, np.float32), np.asarray(bn1_beta, np.float32),
        np.asarray(bn1_mean, np.float32), np.asarray(bn1_var, np.float32),
        np.asarray(bn2_gamma, np.float32), np.asarray(bn2_beta, np.float32),
        np.asarray(bn2_mean, np.float32), np.asarray(bn2_var, np.float32),
        np.asarray(w1, np.float32), np.asarray(w2, np.float32),
        np.asarray(shift, np.float32))

    if "nc" not in _NC_CACHE:
        _NC_CACHE["nc"] = _build_nc()
    nc = _NC_CACHE["nc"]

    NB = x.shape[0]
    xs = x.reshape(N_CORES, N_PER, C, PIX)
    ps = prev_fmap.reshape(N_CORES, N_PER, C, PIX)
    in_maps = [
        {"x": xs[i], "prev": ps[i], **consts}
        for i in range(N_CORES)
    ]

    trace = bool(os.environ.get("CC_KERNEL_TRACE"))
    res = run_bass_kernel_spmd(
        nc, in_maps, core_ids=list(range(N_CORES)), trace=trace,
    )
    LAST_EXEC_NS = res.exec_time_ns

    out = np.empty((NB, C, PIX), np.float32)
    fmap = np.empty((NB, C, PIX), np.float32)
    for i in range(N_CORES):
        out[i * N_PER:(i + 1) * N_PER] = res.results[i]["out"].astype(np.float32)
        fmap[i * N_PER:(i + 1) * N_PER] = res.results[i]["fmap"].astype(np.float32)
    return (out.reshape(NB, C, H, W), fmap.reshape(NB, C, H, W))
